# revision 10
# baseline (speedup 1.0000x reference)
"""Trainium2 Bass kernel for nn_CaC_50637664420271.

Computes, for x:[16,256,64,64]:
  feat_k = wk @ x + bk  (1x1 conv), feat_q = wq @ x + bq
  krnl[n,c,3,3] = bmm(feat_k, feat_q^T)  -> BatchNorm (train stats) ->
  out = mean_d sigmoid(depthwise_conv(x, krnl, dilation=d)), d in {1,2,3}

Sharding: pure data-parallel over batch (2 samples / core, 8 cores), with a
tiny AllReduce of per-channel (sum, sumsq) of krnl for the BN batch stats.

Single-product fp16 pipeline: x is converted to fp16 on the host and loaded
once per unit as a zero-padded image; features stream from the same padded
tile (strided stationary chunks), so x is read from HBM exactly once. The
depthwise conv splits its 9 taps per dilation across the TensorEngine
(diag-weight fp16 matmuls into PSUM), the VectorEngine (tensor_scalar at 4x +
tensor_tensor at 2x on fp16), and the ScalarEngine (copy-with-scale taps);
DVE partials merge into PSUM via identity matmuls, sigmoid reads PSUM
directly, and GPSIMD averages the three sigmoids and writes fp16 output.
"""
import os
import numpy as np

import concourse.bass as bass
import concourse.bacc as bacc
import concourse.tile as tile
import concourse.mybir as mybir
from concourse import bass_utils

N_CORES = 8
NLOC = 2            # samples per core
C = 256
H = W = 64
HW = H * W          # 4096
S = 3
PAD = 3
WP = W + 2 * PAD    # padded row width 70
HP = H + 2 * PAD
PSZ = WP * HP       # 70*70 = 4900 padded image size
CB = C // 128       # channel blocks per sample (2)
NU = NLOC * CB      # units per core (4)
PB = HW // 128      # pixel blocks per sample (32)
FQ = S * S          # 9
FKQ = C + FQ        # 265 fused feature columns
BN_EPS = 1e-5
BN_CNT = 16 * FQ    # 144 elements per channel in BN stats

NQ = 4              # psum quarters per dilation image
QW = HW // NQ       # 1024 px per quarter (16 rows)

# per-dilation engine split of the 9 taps
TAPS = {
    1: {"pe": (0, 2, 4, 6, 8), "dve": (1, 3, 5), "act": (7,)},
    2: {"pe": (0, 2, 4, 6, 8), "dve": (1, 3, 5), "act": (7,)},
    3: {"pe": (0, 2, 4, 6, 8), "dve": (1, 3, 5), "act": (7,)},
}

dt = mybir.dt.float32
f16 = mybir.dt.float16
ALU = mybir.AluOpType
AF = mybir.ActivationFunctionType
AX = mybir.AxisListType


def tap_dydx(t, d):
    return d * (t // S - 1), d * (t % S - 1)


def _body(nc, tc, tens):
    xh_d, w_d, bias_d, g_d, b_d, out_d = tens
    with tc.tile_pool(name="const", bufs=1) as cpool, \
         tc.tile_pool(name="pimg", bufs=4) as ppool, \
         tc.tile_pool(name="fbp", bufs=3) as fbpool, \
         tc.tile_pool(name="tmp", bufs=3) as tpool, \
         tc.tile_pool(name="zd", bufs=2) as zpool, \
         tc.tile_pool(name="ab", bufs=2) as apool, \
         tc.tile_pool(name="sig", bufs=4) as spool, \
         tc.tile_pool(name="acc", bufs=2) as opool, \
         tc.tile_pool(name="diag", bufs=10) as gpool, \
         tc.tile_pool(name="small", bufs=1) as vpool, \
         tc.tile_pool(name="work", bufs=4) as wpool, \
         tc.tile_pool(name="dram", bufs=2, space="DRAM") as dpool:

        # ---- constants / weights ----------------------------------------
        ident_d = nc.inline_tensor(np.eye(128).astype(np.float16),
                                   name="identh")
        ident = cpool.tile([128, 128], f16, tag="ident")
        nc.sync.dma_start(ident[:], ident_d.ap())

        wkq = []
        for ki in range(CB):
            t = cpool.tile([128, FKQ], f16, tag=f"wkq{ki}", name=f"wkq{ki}")
            nc.sync.dma_start(t[:], w_d.ap()[ki * 128:(ki + 1) * 128, :])
            wkq.append(t)

        ones_r = cpool.tile([1, 128], f16, tag="ones")
        nc.vector.memset(ones_r[:], 1.0)
        bias_r = cpool.tile([1, FKQ], dt, tag="biasr")
        nc.sync.dma_start(bias_r[:], bias_d.ap().rearrange("(p f) -> p f", p=1))
        bias_rh = cpool.tile([1, FKQ], f16, tag="biasrh")
        nc.vector.tensor_copy(bias_rh[:], bias_r[:])

        gam, bet = [], []
        for cb in range(CB):
            gt = cpool.tile([128, 1], dt, tag=f"g{cb}", name=f"g{cb}")
            bt = cpool.tile([128, 1], dt, tag=f"b{cb}", name=f"b{cb}")
            nc.sync.dma_start(
                gt[:], g_d.ap().rearrange("(p f) -> p f", f=1)[cb * 128:(cb + 1) * 128, :])
            nc.sync.dma_start(
                bt[:], b_d.ap().rearrange("(p f) -> p f", f=1)[cb * 128:(cb + 1) * 128, :])
            gam.append(gt)
            bet.append(bt)

        def pwin(pt, r0, nr, dy, dx):
            g = pt[:].rearrange("p (r c) -> p r c", c=WP)
            return g[:, PAD + r0 + dy:PAD + r0 + dy + nr,
                     PAD + dx:PAD + dx + W]

        # ---- images: contiguous copies first (feature stationaries, so
        # features can start early), padded fp16 images for conv windows --
        xs = {}
        for u in range(NU):
            s, cb = divmod(u, CB)
            xt = ppool.tile([128, HW], f16, tag="xs", name=f"xs{u}")
            nc.sync.dma_start(
                xt[:], xh_d.ap()[s, cb * 128:(cb + 1) * 128].rearrange(
                    "p r c -> p (r c)"))
            xs[(s, cb)] = xt
        pads = {}
        for u in range(NU):
            s, cb = divmod(u, CB)
            t = ppool.tile([128, PSZ], f16, tag="pimg", name=f"pad{u}")
            pg = t[:].rearrange("p (r c) -> p r c", c=WP)
            nc.gpsimd.memset(t[:, 0:PAD * WP + PAD], 0.0)
            nc.gpsimd.memset(t[:, PSZ - PAD * WP - PAD:PSZ], 0.0)
            nc.gpsimd.memset(pg[:, PAD:PAD + H, 0:PAD], 0.0)
            nc.gpsimd.memset(pg[:, PAD:PAD + H, PAD + W:WP], 0.0)
            for q in range(2):
                rr = H // 2
                nc.scalar.dma_start(
                    pg[:, PAD + q * rr:PAD + (q + 1) * rr, PAD:PAD + W],
                    xh_d.ap()[s, cb * 128:(cb + 1) * 128, q * rr:(q + 1) * rr])
            pads[(s, cb)] = t

        # ---- bias broadcast [128, 265] fp16 via ones-matmul --------------
        with tc.tile_pool(name="psf", bufs=2, space="PSUM") as psf, \
             tc.tile_pool(name="psk", bufs=1, space="PSUM") as psk:
            # ---- features + per-sample kernel bmm -----------------------
            # fbT[p, c'] = sum_c x[c,p] wkq[c,c'] + bias  (pixel-major)
            # krnl[c, t] = sum_p fbT[p, c] * fbT[p, 256+t]
            krnl = [[vpool.tile([128, FQ], dt, tag=f"krnl{s}{cb}",
                                name=f"krnl{s}{cb}")
                     for cb in range(CB)] for s in range(NLOC)]
            for s in range(NLOC):
                kps = [psk.tile([128, FQ], dt, tag=f"kp{cb}", name=f"kp{s}{cb}")
                       for cb in range(CB)]
                fbs = [None] * PB

                def bmm(pb):
                    fb = fbs[pb]
                    for cb in range(CB):
                        nc.tensor.matmul(kps[cb][:],
                                         fb[:, cb * 128:(cb + 1) * 128],
                                         fb[:, C:C + FQ],
                                         start=(pb == 0), stop=(pb == PB - 1))

                for pb in range(PB):
                    fp = psf.tile([128, FKQ], dt, tag="fb", name=f"feat{s}{pb}")
                    for cb in range(CB):
                        stat = xs[(s, cb)][:, pb * 128:(pb + 1) * 128]
                        nc.tensor.matmul(fp[:], stat, wkq[cb][:],
                                         start=(cb == 0), stop=False)
                    # fold biases in exactly: += ones^T @ [bk|bq]
                    nc.tensor.matmul(fp[:], ones_r[:], bias_rh[:],
                                     start=False, stop=True)
                    fb = fbpool.tile([128, FKQ], f16, tag="fbs",
                                     name=f"fb{s}{pb}")
                    if pb % 2 == 0:
                        nc.vector.tensor_copy(fb[:], fp[:])
                    else:
                        nc.scalar.copy(fb[:], fp[:])
                    fbs[pb] = fb
                    # bmm lags one block so the evac round-trip never
                    # stalls the PE
                    if pb >= 1:
                        bmm(pb - 1)
                bmm(PB - 1)
                for cb in range(CB):
                    nc.vector.tensor_copy(krnl[s][cb][:], kps[cb][:])

            # ---- BN stats + AllReduce -----------------------------------
            loc = []
            for cb in range(CB):
                st = vpool.tile([128, 2], dt, tag=f"st{cb}", name=f"st{cb}")
                tmp = wpool.tile([128, FQ], dt, tag="sq", name="sq")
                prt = wpool.tile([128, 4], dt, tag="prt", name="prt")
                for s in range(NLOC):
                    nc.vector.tensor_reduce(prt[:, s:s + 1], krnl[s][cb][:],
                                            AX.X, ALU.add)
                    nc.vector.tensor_tensor(out=tmp[:], in0=krnl[s][cb][:],
                                            in1=krnl[s][cb][:], op=ALU.mult)
                    nc.vector.tensor_reduce(prt[:, 2 + s:3 + s], tmp[:],
                                            AX.X, ALU.add)
                nc.vector.tensor_tensor(out=st[:, 0:1], in0=prt[:, 0:1],
                                        in1=prt[:, 1:2], op=ALU.add)
                nc.vector.tensor_tensor(out=st[:, 1:2], in0=prt[:, 2:3],
                                        in1=prt[:, 3:4], op=ALU.add)
                loc.append(st)

            ib = dpool.tile([CB, 128, 2], dt)
            ob = dpool.tile([CB, 128, 2], dt)
            for cb in range(CB):
                nc.gpsimd.dma_start(ib[cb], loc[cb][:])
            if os.environ.get("PROF_NO_CC"):
                nc.gpsimd.dma_start(ob[:], ib[:])
            else:
                nc.gpsimd.collective_compute(
                    "AllReduce", ALU.add,
                    replica_groups=[list(range(N_CORES))],
                    ins=[ib.opt()], outs=[ob.opt()])

            eps_t = vpool.tile([128, 1], dt, tag="eps")
            nc.vector.memset(eps_t[:], BN_EPS)
            scale, shift = [], []
            for cb in range(CB):
                gl = vpool.tile([128, 2], dt, tag=f"gl{cb}", name=f"gl{cb}")
                nc.gpsimd.dma_start(gl[:], ob[cb])
                mean = wpool.tile([128, 1], dt, tag="mean", name="mean")
                sc = vpool.tile([128, 1], dt, tag=f"sc{cb}", name=f"sc{cb}")
                sh = vpool.tile([128, 1], dt, tag=f"sh{cb}", name=f"sh{cb}")
                t0 = wpool.tile([128, 1], dt, tag="bn0", name="bn0")
                t1 = wpool.tile([128, 1], dt, tag="bn1", name="bn1")
                nc.vector.tensor_scalar_mul(mean[:], gl[:, 0:1], 1.0 / BN_CNT)
                nc.vector.tensor_tensor(out=t0[:], in0=mean[:], in1=mean[:],
                                        op=ALU.mult)
                nc.vector.scalar_tensor_tensor(
                    out=t1[:], in0=gl[:, 1:2], scalar=1.0 / BN_CNT, in1=t0[:],
                    op0=ALU.mult, op1=ALU.subtract)
                nc.scalar.activation(t0[:], t1[:], AF.Sqrt, bias=eps_t[:])
                nc.vector.reciprocal(t1[:], t0[:])
                nc.vector.tensor_tensor(out=sc[:], in0=gam[cb][:], in1=t1[:],
                                        op=ALU.mult)
                nc.vector.tensor_tensor(out=t0[:], in0=mean[:], in1=sc[:],
                                        op=ALU.mult)
                nc.vector.tensor_tensor(out=sh[:], in0=bet[cb][:], in1=t0[:],
                                        op=ALU.subtract)
                scale.append(sc)
                shift.append(sh)

            # normalized per-tap weights w = krnl*scale + shift, fp32
            wnorm = [[None] * CB for _ in range(NLOC)]
            for s in range(NLOC):
                for cb in range(CB):
                    wn = vpool.tile([128, FQ], dt, tag=f"wn{s}{cb}",
                                    name=f"wn{s}{cb}")
                    nc.vector.tensor_scalar(
                        out=wn[:], in0=krnl[s][cb][:],
                        scalar1=scale[cb][:], scalar2=shift[cb][:],
                        op0=ALU.mult, op1=ALU.add)
                    wnorm[s][cb] = wn

        # ---- depthwise convs + sigmoid + average ------------------------
        with tc.tile_pool(name="psz", bufs=4, space="PSUM") as psz:
            for u in range(NU):
                s, cb = divmod(u, CB)
                wn = wnorm[s][cb]
                pad = pads[(s, cb)]
                pe_taps = sorted(set(t for d in (1, 2, 3)
                                     for t in TAPS[d]["pe"]))
                diag = {}
                for t in pe_taps:
                    dg = gpool.tile([128, 128], f16, tag="diag",
                                    name=f"dg{u}_{t}")
                    nc.vector.tensor_scalar_mul(dg[:], ident[:],
                                                wn[:, t:t + 1])
                    diag[t] = dg
                sigs = []
                for d in (1, 2, 3):
                    cfg = TAPS[d]
                    # Act taps -> abuf
                    abufs = []
                    for t in cfg["act"]:
                        dy, dx = tap_dydx(t, d)
                        ab = apool.tile([128, HW], f16, tag="ab",
                                        name=f"ab{u}_{d}_{t}")
                        nc.scalar.activation(
                            ab[:].rearrange("p (r c) -> p r c", c=W),
                            pwin(pad, 0, H, dy, dx), AF.Copy,
                            scale=wn[:, t:t + 1])
                        abufs.append(ab)
                    # DVE taps: TS tmps, then TT chain into zd
                    zd = None
                    if cfg["dve"] or abufs:
                        tmps = []
                        for t in cfg["dve"]:
                            dy, dx = tap_dydx(t, d)
                            tm = tpool.tile([128, HW], f16, tag="tmp",
                                            name=f"tm{u}_{d}_{t}")
                            nc.vector.tensor_scalar_mul(
                                tm[:].rearrange("p (r c) -> p r c", c=W),
                                pwin(pad, 0, H, dy, dx), wn[:, t:t + 1])
                            tmps.append(tm)
                        terms = tmps + abufs
                        zd = zpool.tile([128, HW], f16, tag="zd",
                                        name=f"zd{u}_{d}")
                        nc.vector.tensor_tensor(out=zd[:], in0=terms[0][:],
                                                in1=terms[1][:], op=ALU.add)
                        for term in terms[2:]:
                            nc.vector.tensor_tensor(out=zd[:], in0=zd[:],
                                                    in1=term[:], op=ALU.add)
                    # PE taps + merge into PSUM quarters, sigmoid from PSUM
                    sg = spool.tile([128, HW], f16, tag="sig",
                                    name=f"sg{u}_{d}")
                    for q in range(NQ):
                        r0 = q * (H // NQ)
                        zq = psz.tile([128, QW], dt, tag="z",
                                      name=f"z{u}_{d}_{q}")
                        # per 512-col window: its own start/stop group
                        n_grp = len(cfg["pe"]) + (1 if zd is not None else 0)
                        for ti, t in enumerate(cfg["pe"]):
                            dy, dx = tap_dydx(t, d)
                            for hh in range(2):
                                nc.tensor.matmul(
                                    zq[:, hh * 512:(hh + 1) * 512], diag[t][:],
                                    pwin(pad, r0 + hh * 8, 8, dy, dx),
                                    start=(ti == 0), stop=(ti == n_grp - 1))
                        if zd is not None:
                            for hh in range(2):
                                nc.tensor.matmul(
                                    zq[:, hh * 512:(hh + 1) * 512], ident[:],
                                    zd[:, q * QW + hh * 512:
                                       q * QW + (hh + 1) * 512],
                                    start=False, stop=True)
                        nc.scalar.activation(sg[:, q * QW:(q + 1) * QW],
                                             zq[:], AF.Sigmoid)
                    sigs.append(sg)
                # average of the three sigmoids, fp16 out. GPSIMD for the
                # pipelined units; DVE for the last one (shorter drain tail)
                acc = opool.tile([128, HW], f16, tag="acc", name=f"acc{u}")
                outb = opool.tile([128, HW], f16, tag="outb", name=f"outb{u}")
                if u < NU - 1:
                    nc.gpsimd.tensor_tensor(out=acc[:], in0=sigs[0][:],
                                            in1=sigs[1][:], op=ALU.add)
                    nc.gpsimd.tensor_tensor(out=acc[:], in0=acc[:],
                                            in1=sigs[2][:], op=ALU.add)
                    nc.gpsimd.tensor_scalar_mul(outb[:], acc[:], 1.0 / 3.0)
                else:
                    nc.vector.tensor_tensor(out=acc[:], in0=sigs[0][:],
                                            in1=sigs[1][:], op=ALU.add)
                    nc.vector.tensor_tensor(out=acc[:], in0=acc[:],
                                            in1=sigs[2][:], op=ALU.add)
                    nc.vector.tensor_scalar_mul(outb[:], acc[:], 1.0 / 3.0)
                nc.gpsimd.dma_start(
                    out_d.ap()[s, cb * 128:(cb + 1) * 128],
                    outb[:].rearrange("p (r c) -> p r c", c=W))


def _build():
    nc = bacc.Bacc("TRN2", debug=False, num_devices=N_CORES,
                   target_bir_lowering=False)
    xh_d = nc.dram_tensor("xh", [NLOC, C, H, W], f16, kind="ExternalInput")
    w_d = nc.dram_tensor("wkqt", [C, FKQ], f16, kind="ExternalInput")
    bias_d = nc.dram_tensor("biaskq", [FKQ], dt, kind="ExternalInput")
    g_d = nc.dram_tensor("gamma", [C], dt, kind="ExternalInput")
    b_d = nc.dram_tensor("beta", [C], dt, kind="ExternalInput")
    out_d = nc.dram_tensor("out", [NLOC, C, H, W], f16, kind="ExternalOutput")
    with tile.TileContext(nc) as tc:
        _body(nc, tc, (xh_d, w_d, bias_d, g_d, b_d, out_d))
    nc.compile()
    return nc


_nc_cache = None
last_results = None


def kernel(x, wk, bk, wq, bq, gamma, beta):
    global _nc_cache, last_results
    if _nc_cache is None:
        _nc_cache = _build()
    nc = _nc_cache
    x = np.ascontiguousarray(x, dtype=np.float32)
    xh = x.astype(np.float16)
    wkqt = np.concatenate(
        [np.asarray(wk, np.float32).T, np.asarray(wq, np.float32).T],
        axis=1).astype(np.float16)  # [C, 265]
    biaskq = np.concatenate(
        [np.asarray(bk, np.float32), np.asarray(bq, np.float32)])
    in_maps = []
    for c in range(N_CORES):
        sl = slice(c * NLOC, (c + 1) * NLOC)
        in_maps.append({
            "xh": np.ascontiguousarray(xh[sl]),
            "wkqt": np.ascontiguousarray(wkqt),
            "biaskq": np.ascontiguousarray(biaskq, np.float32),
            "gamma": np.ascontiguousarray(gamma, np.float32),
            "beta": np.ascontiguousarray(beta, np.float32),
        })
    res = bass_utils.run_bass_kernel_spmd(
        nc, in_maps, core_ids=list(range(N_CORES)))
    last_results = res
    out = np.concatenate([res.results[c]["out"] for c in range(N_CORES)],
                         axis=0)
    return out.astype(np.float32)


# revision 16
# speedup vs baseline: 1.1863x; 1.1863x over previous
"""Trainium2 Bass kernel for nn_CaC_50637664420271.

Computes, for x:[16,256,64,64]:
  feat_k = wk @ x + bk  (1x1 conv), feat_q = wq @ x + bq
  krnl[n,c,3,3] = bmm(feat_k, feat_q^T)  -> BatchNorm (train stats) ->
  out = mean_d sigmoid(depthwise_conv(x, krnl, dilation=d)), d in {1,2,3}

Sharding: pure data-parallel over batch (2 samples / core, 8 cores), with a
tiny AllReduce of per-channel (sum, sumsq) of krnl for the BN batch stats.

Single-product fp16 pipeline: x is converted to fp16 on the host and loaded
once per unit as a zero-padded image; features stream from the same padded
tile (strided stationary chunks), so x is read from HBM exactly once. The
depthwise conv splits its 9 taps per dilation across the TensorEngine
(diag-weight fp16 matmuls into PSUM), the VectorEngine (tensor_scalar at 4x +
tensor_tensor at 2x on fp16), and the ScalarEngine (copy-with-scale taps);
DVE partials merge into PSUM via identity matmuls, sigmoid reads PSUM
directly, and GPSIMD averages the three sigmoids and writes fp16 output.
"""
import os
import numpy as np

import concourse.bass as bass
import concourse.bacc as bacc
import concourse.tile as tile
import concourse.mybir as mybir
from concourse import bass_utils

N_CORES = 8
NLOC = 2            # samples per core
C = 256
H = W = 64
HW = H * W          # 4096
S = 3
PAD = 3
WP = W + 2 * PAD    # padded row width 70
HP = H + 2 * PAD
PSZ = WP * HP       # 70*70 = 4900 padded image size
CB = C // 128       # channel blocks per sample (2)
NU = NLOC * CB      # units per core (4)
PB = HW // 128      # pixel blocks per sample (32)
FQ = S * S          # 9
FKQ = C + FQ        # 265 fused feature columns
BN_EPS = 1e-5
BN_CNT = 16 * FQ    # 144 elements per channel in BN stats

NQ = 4              # psum quarters per dilation image
QW = HW // NQ       # 1024 px per quarter (16 rows)

# per-dilation engine split of the 9 taps
TAPS = {
    1: {"pe": (0, 2, 4, 6, 8), "dve": (1, 3, 5), "act": (7,)},
    2: {"pe": (0, 2, 4, 6, 8), "dve": (1, 3, 5), "act": (7,)},
    3: {"pe": (0, 2, 4, 6, 8), "dve": (1, 3, 5), "act": (7,)},
}

dt = mybir.dt.float32
f16 = mybir.dt.float16
ALU = mybir.AluOpType
AF = mybir.ActivationFunctionType
AX = mybir.AxisListType


def tap_dydx(t, d):
    return d * (t // S - 1), d * (t % S - 1)


def _body(nc, tc, tens):
    xh_d, w_d, bias_d, g_d, b_d, out_d = tens
    with tc.tile_pool(name="const", bufs=1) as cpool, \
         tc.tile_pool(name="pimg", bufs=4) as ppool, \
         tc.tile_pool(name="fbp", bufs=4) as fbpool, \
         tc.tile_pool(name="tmp", bufs=3) as tpool, \
         tc.tile_pool(name="zd", bufs=2) as zpool, \
         tc.tile_pool(name="ab", bufs=2) as apool, \
         tc.tile_pool(name="sig", bufs=4) as spool, \
         tc.tile_pool(name="acc", bufs=2) as opool, \
         tc.tile_pool(name="diag", bufs=10) as gpool, \
         tc.tile_pool(name="small", bufs=1) as vpool, \
         tc.tile_pool(name="work", bufs=4) as wpool, \
         tc.tile_pool(name="dram", bufs=2, space="DRAM") as dpool:

        # ---- constants / weights ----------------------------------------
        ident_d = nc.inline_tensor(np.eye(128).astype(np.float16),
                                   name="identh")
        ident = cpool.tile([128, 128], f16, tag="ident")
        nc.sync.dma_start(ident[:], ident_d.ap())

        wkq = []
        for ki in range(CB):
            t = cpool.tile([128, FKQ], f16, tag=f"wkq{ki}", name=f"wkq{ki}")
            nc.sync.dma_start(t[:], w_d.ap()[ki * 128:(ki + 1) * 128, :])
            wkq.append(t)

        ones_r = cpool.tile([1, 128], f16, tag="ones")
        nc.vector.memset(ones_r[:], 1.0)
        bias_r = cpool.tile([1, FKQ], dt, tag="biasr")
        nc.sync.dma_start(bias_r[:], bias_d.ap().rearrange("(p f) -> p f", p=1))
        bias_rh = cpool.tile([1, FKQ], f16, tag="biasrh")
        nc.vector.tensor_copy(bias_rh[:], bias_r[:])

        gam, bet = [], []
        for cb in range(CB):
            gt = cpool.tile([128, 1], dt, tag=f"g{cb}", name=f"g{cb}")
            bt = cpool.tile([128, 1], dt, tag=f"b{cb}", name=f"b{cb}")
            nc.sync.dma_start(
                gt[:], g_d.ap().rearrange("(p f) -> p f", f=1)[cb * 128:(cb + 1) * 128, :])
            nc.sync.dma_start(
                bt[:], b_d.ap().rearrange("(p f) -> p f", f=1)[cb * 128:(cb + 1) * 128, :])
            gam.append(gt)
            bet.append(bt)

        def pwin(pt, r0, nr, dy, dx):
            g = pt[:].rearrange("p (r c) -> p r c", c=WP)
            return g[:, PAD + r0 + dy:PAD + r0 + dy + nr,
                     PAD + dx:PAD + dx + W]

        # ---- images: contiguous copies first (feature stationaries, so
        # features can start early), padded fp16 images for conv windows --
        xs = {}
        for u in range(NU):
            s, cb = divmod(u, CB)
            xt = ppool.tile([128, HW], f16, tag="xs", name=f"xs{u}")
            nc.sync.dma_start(
                xt[:], xh_d.ap()[s, cb * 128:(cb + 1) * 128].rearrange(
                    "p r c -> p (r c)"))
            xs[(s, cb)] = xt
        pads = {}
        for u in range(NU):
            s, cb = divmod(u, CB)
            t = ppool.tile([128, PSZ], f16, tag="pimg", name=f"pad{u}")
            pg = t[:].rearrange("p (r c) -> p r c", c=WP)
            nc.gpsimd.memset(t[:, 0:PAD * WP + PAD], 0.0)
            nc.gpsimd.memset(t[:, PSZ - PAD * WP - PAD:PSZ], 0.0)
            nc.gpsimd.memset(pg[:, PAD:PAD + H, 0:PAD], 0.0)
            nc.gpsimd.memset(pg[:, PAD:PAD + H, PAD + W:WP], 0.0)
            pads[(s, cb)] = t
        # pad interiors stream on the same queue after the xs tiles so the
        # feature matmuls (which only need xs) start as early as possible
        for u in range(NU):
            s, cb = divmod(u, CB)
            pg = pads[(s, cb)][:].rearrange("p (r c) -> p r c", c=WP)
            for q in range(2):
                rr = H // 2
                nc.sync.dma_start(
                    pg[:, PAD + q * rr:PAD + (q + 1) * rr, PAD:PAD + W],
                    xh_d.ap()[s, cb * 128:(cb + 1) * 128, q * rr:(q + 1) * rr])

        # ---- bias broadcast [128, 265] fp16 via ones-matmul --------------
        with tc.tile_pool(name="psf", bufs=4, space="PSUM") as psf, \
             tc.tile_pool(name="psk", bufs=1, space="PSUM") as psk:
            # ---- features + per-sample kernel bmm -----------------------
            # fbT[p, c'] = sum_c x[c,p] wkq[c,c'] + bias  (pixel-major)
            # krnl[c, t] = sum_p fbT[p, c] * fbT[p, 256+t]
            krnl = [[vpool.tile([128, FQ], dt, tag=f"krnl{s}{cb}",
                                name=f"krnl{s}{cb}")
                     for cb in range(CB)] for s in range(NLOC)]
            prts = [vpool.tile([128, 4], dt, tag=f"prt{cb}", name=f"prt{cb}")
                    for cb in range(CB)]
            for s in range(NLOC):
                kps = [psk.tile([128, FQ], dt, tag=f"kp{cb}", name=f"kp{s}{cb}")
                       for cb in range(CB)]
                fbs = [None] * PB

                def bmm(pb):
                    fb = fbs[pb]
                    for cb in range(CB):
                        nc.tensor.matmul(kps[cb][:],
                                         fb[:, cb * 128:(cb + 1) * 128],
                                         fb[:, C:C + FQ],
                                         start=(pb == 0), stop=(pb == PB - 1))

                for pb in range(PB):
                    fp = psf.tile([128, FKQ], dt, tag="fb", name=f"feat{s}{pb}")
                    for cb in range(CB):
                        stat = xs[(s, cb)][:, pb * 128:(pb + 1) * 128]
                        nc.tensor.matmul(fp[:], stat, wkq[cb][:],
                                         start=(cb == 0), stop=False)
                    # fold biases in exactly: += ones^T @ [bk|bq]
                    nc.tensor.matmul(fp[:], ones_r[:], bias_rh[:],
                                     start=False, stop=True)
                    fb = fbpool.tile([128, FKQ], f16, tag="fbs",
                                     name=f"fb{s}{pb}")
                    if pb % 2 == 0:
                        nc.vector.tensor_copy(fb[:], fp[:])
                    else:
                        nc.scalar.copy(fb[:], fp[:])
                    fbs[pb] = fb
                    # bmm lags one block so the evac round-trip never
                    # stalls the PE
                    if pb >= 1:
                        bmm(pb - 1)
                bmm(PB - 1)
                for cb in range(CB):
                    nc.vector.tensor_copy(krnl[s][cb][:], kps[cb][:])
                    # per-sample stat partials overlap the next sample
                    nc.vector.tensor_reduce(prts[cb][:, s:s + 1],
                                            krnl[s][cb][:], AX.X, ALU.add)
                    tmp = wpool.tile([128, FQ], dt, tag="sq", name="sq")
                    nc.vector.tensor_tensor(out=tmp[:], in0=krnl[s][cb][:],
                                            in1=krnl[s][cb][:], op=ALU.mult)
                    nc.vector.tensor_reduce(prts[cb][:, 2 + s:3 + s], tmp[:],
                                            AX.X, ALU.add)

            # ---- BN stats + AllReduce -----------------------------------
            loc = []
            for cb in range(CB):
                st = vpool.tile([128, 2], dt, tag=f"st{cb}", name=f"st{cb}")
                nc.vector.tensor_tensor(out=st[:, 0:1], in0=prts[cb][:, 0:1],
                                        in1=prts[cb][:, 1:2], op=ALU.add)
                nc.vector.tensor_tensor(out=st[:, 1:2], in0=prts[cb][:, 2:3],
                                        in1=prts[cb][:, 3:4], op=ALU.add)
                loc.append(st)

            ib = dpool.tile([CB, 128, 2], dt)
            ob = dpool.tile([CB, 128, 2], dt)
            for cb in range(CB):
                nc.gpsimd.dma_start(ib[cb], loc[cb][:])
            if os.environ.get("PROF_NO_CC"):
                nc.gpsimd.dma_start(ob[:], ib[:])
            else:
                nc.gpsimd.collective_compute(
                    "AllReduce", ALU.add,
                    replica_groups=[list(range(N_CORES))],
                    ins=[ib.opt()], outs=[ob.opt()])

            eps_t = vpool.tile([128, 1], dt, tag="eps")
            nc.vector.memset(eps_t[:], BN_EPS)
            scale, shift = [], []
            for cb in range(CB):
                gl = vpool.tile([128, 2], dt, tag=f"gl{cb}", name=f"gl{cb}")
                nc.gpsimd.dma_start(gl[:], ob[cb])
                mean = wpool.tile([128, 1], dt, tag="mean", name="mean")
                sc = vpool.tile([128, 1], dt, tag=f"sc{cb}", name=f"sc{cb}")
                sh = vpool.tile([128, 1], dt, tag=f"sh{cb}", name=f"sh{cb}")
                t0 = wpool.tile([128, 1], dt, tag="bn0", name="bn0")
                t1 = wpool.tile([128, 1], dt, tag="bn1", name="bn1")
                nc.vector.tensor_scalar_mul(mean[:], gl[:, 0:1], 1.0 / BN_CNT)
                nc.vector.tensor_tensor(out=t0[:], in0=mean[:], in1=mean[:],
                                        op=ALU.mult)
                nc.vector.scalar_tensor_tensor(
                    out=t1[:], in0=gl[:, 1:2], scalar=1.0 / BN_CNT, in1=t0[:],
                    op0=ALU.mult, op1=ALU.subtract)
                nc.scalar.activation(t0[:], t1[:], AF.Sqrt, bias=eps_t[:])
                nc.vector.reciprocal(t1[:], t0[:])
                nc.vector.tensor_tensor(out=sc[:], in0=gam[cb][:], in1=t1[:],
                                        op=ALU.mult)
                nc.vector.tensor_tensor(out=t0[:], in0=mean[:], in1=sc[:],
                                        op=ALU.mult)
                nc.vector.tensor_tensor(out=sh[:], in0=bet[cb][:], in1=t0[:],
                                        op=ALU.subtract)
                scale.append(sc)
                shift.append(sh)

            # normalized per-tap weights w = krnl*scale + shift, fp32
            wnorm = [[None] * CB for _ in range(NLOC)]
            for s in range(NLOC):
                for cb in range(CB):
                    wn = vpool.tile([128, FQ], dt, tag=f"wn{s}{cb}",
                                    name=f"wn{s}{cb}")
                    nc.vector.tensor_scalar(
                        out=wn[:], in0=krnl[s][cb][:],
                        scalar1=scale[cb][:], scalar2=shift[cb][:],
                        op0=ALU.mult, op1=ALU.add)
                    wnorm[s][cb] = wn

        # ---- depthwise convs + sigmoid + average ------------------------
        with tc.tile_pool(name="psz", bufs=4, space="PSUM") as psz:
            for u in range(NU):
                s, cb = divmod(u, CB)
                wn = wnorm[s][cb]
                pad = pads[(s, cb)]
                pe_taps = sorted(set(t for d in (1, 2, 3)
                                     for t in TAPS[d]["pe"]))
                diag = {}
                for t in pe_taps:
                    dg = gpool.tile([128, 128], f16, tag="diag",
                                    name=f"dg{u}_{t}")
                    nc.vector.tensor_scalar_mul(dg[:], ident[:],
                                                wn[:, t:t + 1])
                    diag[t] = dg
                sigs = []
                for d in (1, 2, 3):
                    cfg = TAPS[d]
                    # Act taps -> abuf
                    abufs = []
                    for t in cfg["act"]:
                        dy, dx = tap_dydx(t, d)
                        ab = apool.tile([128, HW], f16, tag="ab",
                                        name=f"ab{u}_{d}_{t}")
                        nc.scalar.activation(
                            ab[:].rearrange("p (r c) -> p r c", c=W),
                            pwin(pad, 0, H, dy, dx), AF.Copy,
                            scale=wn[:, t:t + 1])
                        abufs.append(ab)
                    # DVE taps: TS tmps, then TT chain into zd
                    zd = None
                    if cfg["dve"] or abufs:
                        tmps = []
                        for t in cfg["dve"]:
                            dy, dx = tap_dydx(t, d)
                            tm = tpool.tile([128, HW], f16, tag="tmp",
                                            name=f"tm{u}_{d}_{t}")
                            nc.vector.tensor_scalar_mul(
                                tm[:].rearrange("p (r c) -> p r c", c=W),
                                pwin(pad, 0, H, dy, dx), wn[:, t:t + 1])
                            tmps.append(tm)
                        terms = tmps + abufs
                        zd = zpool.tile([128, HW], f16, tag="zd",
                                        name=f"zd{u}_{d}")
                        nc.vector.tensor_tensor(out=zd[:], in0=terms[0][:],
                                                in1=terms[1][:], op=ALU.add)
                        for term in terms[2:]:
                            nc.vector.tensor_tensor(out=zd[:], in0=zd[:],
                                                    in1=term[:], op=ALU.add)
                    # PE taps + merge into PSUM quarters, sigmoid from PSUM
                    sg = spool.tile([128, HW], f16, tag="sig",
                                    name=f"sg{u}_{d}")
                    for q in range(NQ):
                        r0 = q * (H // NQ)
                        zq = psz.tile([128, QW], dt, tag="z",
                                      name=f"z{u}_{d}_{q}")
                        # per 512-col window: its own start/stop group
                        n_grp = len(cfg["pe"]) + (1 if zd is not None else 0)
                        for ti, t in enumerate(cfg["pe"]):
                            dy, dx = tap_dydx(t, d)
                            for hh in range(2):
                                nc.tensor.matmul(
                                    zq[:, hh * 512:(hh + 1) * 512], diag[t][:],
                                    pwin(pad, r0 + hh * 8, 8, dy, dx),
                                    start=(ti == 0), stop=(ti == n_grp - 1))
                        if zd is not None:
                            for hh in range(2):
                                nc.tensor.matmul(
                                    zq[:, hh * 512:(hh + 1) * 512], ident[:],
                                    zd[:, q * QW + hh * 512:
                                       q * QW + (hh + 1) * 512],
                                    start=False, stop=True)
                        nc.scalar.activation(sg[:, q * QW:(q + 1) * QW],
                                             zq[:], AF.Sigmoid)
                    sigs.append(sg)
                # average of the three sigmoids, fp16 out. GPSIMD for the
                # pipelined units; DVE for the last one (shorter drain tail)
                acc = opool.tile([128, HW], f16, tag="acc", name=f"acc{u}")
                outb = opool.tile([128, HW], f16, tag="outb", name=f"outb{u}")
                if u < NU - 1:
                    nc.gpsimd.tensor_tensor(out=acc[:], in0=sigs[0][:],
                                            in1=sigs[1][:], op=ALU.add)
                    nc.gpsimd.tensor_tensor(out=acc[:], in0=acc[:],
                                            in1=sigs[2][:], op=ALU.add)
                    nc.gpsimd.tensor_scalar_mul(outb[:], acc[:], 1.0 / 3.0)
                    nc.gpsimd.dma_start(
                        out_d.ap()[s, cb * 128:(cb + 1) * 128],
                        outb[:].rearrange("p (r c) -> p r c", c=W))
                else:
                    # last unit: quarter-chunked DVE average + eager DMA,
                    # so the drain tail is one quarter, not a full image
                    for q in range(NQ):
                        ql = slice(q * QW, (q + 1) * QW)
                        nc.vector.tensor_tensor(out=acc[:, ql],
                                                in0=sigs[0][:, ql],
                                                in1=sigs[1][:, ql], op=ALU.add)
                        nc.vector.tensor_tensor(out=acc[:, ql],
                                                in0=acc[:, ql],
                                                in1=sigs[2][:, ql], op=ALU.add)
                        nc.vector.tensor_scalar_mul(outb[:, ql], acc[:, ql],
                                                    1.0 / 3.0)
                        nc.gpsimd.dma_start(
                            out_d.ap()[s, cb * 128:(cb + 1) * 128,
                                       q * 16:(q + 1) * 16],
                            outb[:, ql].rearrange("p (r c) -> p r c", c=W))


def _build():
    nc = bacc.Bacc("TRN2", debug=False, num_devices=N_CORES,
                   target_bir_lowering=False)
    xh_d = nc.dram_tensor("xh", [NLOC, C, H, W], f16, kind="ExternalInput")
    w_d = nc.dram_tensor("wkqt", [C, FKQ], f16, kind="ExternalInput")
    bias_d = nc.dram_tensor("biaskq", [FKQ], dt, kind="ExternalInput")
    g_d = nc.dram_tensor("gamma", [C], dt, kind="ExternalInput")
    b_d = nc.dram_tensor("beta", [C], dt, kind="ExternalInput")
    out_d = nc.dram_tensor("out", [NLOC, C, H, W], f16, kind="ExternalOutput")
    with tile.TileContext(nc) as tc:
        _body(nc, tc, (xh_d, w_d, bias_d, g_d, b_d, out_d))
    nc.compile()
    return nc


_nc_cache = None
last_results = None


def kernel(x, wk, bk, wq, bq, gamma, beta):
    global _nc_cache, last_results
    if _nc_cache is None:
        _nc_cache = _build()
    nc = _nc_cache
    x = np.ascontiguousarray(x, dtype=np.float32)
    xh = x.astype(np.float16)
    wkqt = np.concatenate(
        [np.asarray(wk, np.float32).T, np.asarray(wq, np.float32).T],
        axis=1).astype(np.float16)  # [C, 265]
    biaskq = np.concatenate(
        [np.asarray(bk, np.float32), np.asarray(bq, np.float32)])
    in_maps = []
    for c in range(N_CORES):
        sl = slice(c * NLOC, (c + 1) * NLOC)
        in_maps.append({
            "xh": np.ascontiguousarray(xh[sl]),
            "wkqt": np.ascontiguousarray(wkqt),
            "biaskq": np.ascontiguousarray(biaskq, np.float32),
            "gamma": np.ascontiguousarray(gamma, np.float32),
            "beta": np.ascontiguousarray(beta, np.float32),
        })
    res = bass_utils.run_bass_kernel_spmd(
        nc, in_maps, core_ids=list(range(N_CORES)))
    last_results = res
    out = np.concatenate([res.results[c]["out"] for c in range(N_CORES)],
                         axis=0)
    return out.astype(np.float32)


# revision 44
# speedup vs baseline: 1.2137x; 1.0231x over previous
"""Trainium2 Bass kernel for nn_CaC_50637664420271.

Computes, for x:[16,256,64,64]:
  feat_k = wk @ x + bk  (1x1 conv), feat_q = wq @ x + bq
  krnl[n,c,3,3] = bmm(feat_k, feat_q^T)  -> BatchNorm (train stats) ->
  out = mean_d sigmoid(depthwise_conv(x, krnl, dilation=d)), d in {1,2,3}

Sharding: pure data-parallel over batch (2 samples / core, 8 cores), with a
tiny AllReduce of per-channel (sum, sumsq) of krnl for the BN batch stats.

Single-product fp16 pipeline: x is converted to fp16 on the host and loaded
once per unit as a zero-padded image; features stream from the same padded
tile (strided stationary chunks), so x is read from HBM exactly once. The
depthwise conv splits its 9 taps per dilation across the TensorEngine
(diag-weight fp16 matmuls into PSUM), the VectorEngine (tensor_scalar at 4x +
tensor_tensor at 2x on fp16), and the ScalarEngine (copy-with-scale taps);
DVE partials merge into PSUM via identity matmuls, sigmoid reads PSUM
directly, and GPSIMD averages the three sigmoids and writes fp16 output.
"""
import os
import numpy as np

import concourse.bass as bass
import concourse.bacc as bacc
import concourse.tile as tile
import concourse.mybir as mybir
from concourse import bass_utils

N_CORES = 8
NLOC = 2            # samples per core
C = 256
H = W = 64
HW = H * W          # 4096
S = 3
PAD = 3
WP = W + 2 * PAD    # padded row width 70
HP = H + 2 * PAD
PSZ = WP * HP       # 70*70 = 4900 padded image size
CB = C // 128       # channel blocks per sample (2)
NU = NLOC * CB      # units per core (4)
PB = HW // 128      # pixel blocks per sample (32)
FQ = S * S          # 9
FKQ = C + FQ        # 265 fused feature columns
BN_EPS = 1e-5
BN_CNT = 16 * FQ    # 144 elements per channel in BN stats

BIAS_GENERAL = True  # set by _build(); False skips the (always-zero) biases

NQ = 4              # psum quarters per dilation image
QW = HW // NQ       # 1024 px per quarter (16 rows)

# per-dilation engine split of the 9 taps; center (4) first so every PSUM
# window's start-tap has full row coverage (shifted taps trim zero rows)
TAPS = {
    1: {"pe": (4, 0, 2, 6, 8), "dve": (1, 3, 5), "act": (7,)},
    2: {"pe": (4, 0, 2, 6, 8), "dve": (1, 3, 5), "act": (7,)},
    3: {"pe": (4, 0, 2, 6, 8), "dve": (1, 3, 5), "act": (7,)},
}

dt = mybir.dt.float32
f16 = mybir.dt.float16
ALU = mybir.AluOpType
AF = mybir.ActivationFunctionType
AX = mybir.AxisListType


def tap_dydx(t, d):
    return d * (t // S - 1), d * (t % S - 1)


def _body(nc, tc, tens):
    xh_d, w_d, bias_d, g_d, b_d, out_d = tens
    with tc.tile_pool(name="const", bufs=1) as cpool, \
         tc.tile_pool(name="pimg", bufs=4) as ppool, \
         tc.tile_pool(name="fbp", bufs=6) as fbpool, \
         tc.tile_pool(name="tmp", bufs=3) as tpool, \
         tc.tile_pool(name="zd", bufs=2) as zpool, \
         tc.tile_pool(name="ab", bufs=2) as apool, \
         tc.tile_pool(name="sig", bufs=4) as spool, \
         tc.tile_pool(name="acc", bufs=2) as opool, \
         tc.tile_pool(name="diag", bufs=10) as gpool, \
         tc.tile_pool(name="small", bufs=1) as vpool, \
         tc.tile_pool(name="work", bufs=4) as wpool, \
         tc.tile_pool(name="dram", bufs=2, space="DRAM") as dpool:

        # ---- constants / weights ----------------------------------------
        ident_d = nc.inline_tensor(np.eye(128).astype(np.float16),
                                   name="identh")
        ident = cpool.tile([128, 128], f16, tag="ident")
        nc.sync.dma_start(ident[:], ident_d.ap())

        wkq = []
        for ki in range(CB):
            t = cpool.tile([128, FKQ], f16, tag=f"wkq{ki}", name=f"wkq{ki}")
            nc.sync.dma_start(t[:], w_d.ap()[ki * 128:(ki + 1) * 128, :])
            wkq.append(t)

        ones_r32 = cpool.tile([1, 128], dt, tag="ones32")
        nc.vector.memset(ones_r32[:], 1.0)
        bias_r = cpool.tile([1, FKQ], dt, tag="biasr")
        nc.sync.dma_start(bias_r[:], bias_d.ap().rearrange("(p f) -> p f", p=1))
        gam, bet = [], []
        for cb in range(CB):
            gt = cpool.tile([128, 1], dt, tag=f"g{cb}", name=f"g{cb}")
            bt = cpool.tile([128, 1], dt, tag=f"b{cb}", name=f"b{cb}")
            nc.sync.dma_start(
                gt[:], g_d.ap().rearrange("(p f) -> p f", f=1)[cb * 128:(cb + 1) * 128, :])
            nc.sync.dma_start(
                bt[:], b_d.ap().rearrange("(p f) -> p f", f=1)[cb * 128:(cb + 1) * 128, :])
            gam.append(gt)
            bet.append(bt)

        def pwin(pt, r0, nr, dy, dx):
            g = pt[:].rearrange("p (r c) -> p r c", c=WP)
            return g[:, PAD + r0 + dy:PAD + r0 + dy + nr,
                     PAD + dx:PAD + dx + W]

        # ---- images: contiguous copies first (feature stationaries, so
        # features can start early), padded fp16 images for conv windows --
        xs = {}
        for u in range(NU):
            s, cb = divmod(u, CB)
            xt = ppool.tile([128, HW], f16, tag="xs", name=f"xs{u}")
            for q in range(2):
                nc.sync.dma_start(
                    xt[:, q * (HW // 2):(q + 1) * (HW // 2)],
                    xh_d.ap()[s, cb * 128:(cb + 1) * 128].rearrange(
                        "p r c -> p (r c)")[:, q * (HW // 2):(q + 1) * (HW // 2)])
            xs[(s, cb)] = xt
        pads = {}
        for u in range(NU):
            s, cb = divmod(u, CB)
            t = ppool.tile([128, PSZ], f16, tag="pimg", name=f"pad{u}")
            pg = t[:].rearrange("p (r c) -> p r c", c=WP)
            nc.gpsimd.memset(t[:, 0:PAD * WP + PAD], 0.0)
            nc.gpsimd.memset(t[:, PSZ - PAD * WP - PAD:PSZ], 0.0)
            nc.gpsimd.memset(pg[:, PAD:PAD + H, 0:PAD], 0.0)
            nc.gpsimd.memset(pg[:, PAD:PAD + H, PAD + W:WP], 0.0)
            pads[(s, cb)] = t
            pg2 = t[:].rearrange("p (r c) -> p r c", c=WP)
            for q in range(2):
                rr = H // 2
                nc.sync.dma_start(
                    pg2[:, PAD + q * rr:PAD + (q + 1) * rr, PAD:PAD + W],
                    xh_d.ap()[s, cb * 128:(cb + 1) * 128, q * rr:(q + 1) * rr])

        # ---- bias broadcast [128, 265] fp16 via ones-matmul --------------
        with tc.tile_pool(name="psf", bufs=5, space="PSUM") as psf, \
             tc.tile_pool(name="psk", bufs=1, space="PSUM") as psk:
            # ---- features + per-sample kernel bmm -----------------------
            # fbT[p, c'] = sum_c x[c,p] wkq[c,c'] + bias  (pixel-major)
            # krnl[c, t] = sum_p fbT[p, c] * fbT[p, 256+t]
            krnl = [[vpool.tile([128, FQ], dt, tag=f"krnl{s}{cb}",
                                name=f"krnl{s}{cb}")
                     for cb in range(CB)] for s in range(NLOC)]
            prts = [vpool.tile([128, 4], dt, tag=f"prt{cb}", name=f"prt{cb}")
                    for cb in range(CB)]
            # bias broadcast [128, 265] fp16, folded into the evac TT
            bbp = psk.tile([128, FKQ], dt, tag="bc", name="bbp")
            nc.tensor.matmul(bbp[:], ones_r32[:], bias_r[:],
                             start=True, stop=True)
            bias_bc = cpool.tile([128, FKQ], f16, tag="bbc")
            nc.vector.tensor_copy(bias_bc[:], bbp[:])

            for s in range(NLOC):
                kps = [psk.tile([128, FQ], dt, tag=f"kp{cb}",
                                name=f"kp{s}{cb}")
                       for cb in range(CB)]
                fbs = [None] * PB

                def bmm(pb):
                    fb = fbs[pb]
                    for cb in range(CB):
                        nc.tensor.matmul(kps[cb][:],
                                         fb[:, cb * 128:(cb + 1) * 128],
                                         fb[:, C:C + FQ],
                                         start=(pb == 0), stop=(pb == PB - 1))

                for pb in range(PB):
                    fp = psf.tile([128, FKQ], dt, tag="fb", name=f"feat{s}{pb}")
                    for cb in range(CB):
                        stat = xs[(s, cb)][:, pb * 128:(pb + 1) * 128]
                        nc.tensor.matmul(fp[:], stat, wkq[cb][:],
                                         start=(cb == 0), stop=(cb == CB - 1))
                    fb = fbpool.tile([128, FKQ], f16, tag="fbs",
                                     name=f"fb{s}{pb}")
                    if pb % 2 == 0:
                        nc.vector.tensor_tensor(out=fb[:], in0=fp[:],
                                                in1=bias_bc[:], op=ALU.add)
                    else:
                        nc.scalar.copy(fb[:], fp[:])
                        nc.vector.tensor_tensor(out=fb[:], in0=fb[:],
                                                in1=bias_bc[:], op=ALU.add)
                    fbs[pb] = fb
                    # bmm lags two blocks so the evac round-trip never
                    # stalls the PE
                    if pb >= 2:
                        bmm(pb - 2)
                bmm(PB - 2)
                bmm(PB - 1)
                for cb in range(CB):
                    nc.vector.tensor_copy(krnl[s][cb][:], kps[cb][:])
                    # per-sample stat partials overlap the next sample
                    nc.vector.tensor_reduce(prts[cb][:, s:s + 1],
                                            krnl[s][cb][:], AX.X, ALU.add)
                    tmp = wpool.tile([128, FQ], dt, tag="sq", name="sq")
                    nc.vector.tensor_tensor(out=tmp[:], in0=krnl[s][cb][:],
                                            in1=krnl[s][cb][:], op=ALU.mult)
                    nc.vector.tensor_reduce(prts[cb][:, 2 + s:3 + s], tmp[:],
                                            AX.X, ALU.add)

            # ---- BN stats + AllReduce -----------------------------------
            # single [128, 4] tile: cols (sum0, sq0, sum1, sq1) per cb pair
            stt = vpool.tile([128, 4], dt, tag="stt", name="stt")
            for cb in range(CB):
                nc.vector.tensor_tensor(out=stt[:, 2 * cb:2 * cb + 1],
                                        in0=prts[cb][:, 0:1],
                                        in1=prts[cb][:, 1:2], op=ALU.add)
                nc.vector.tensor_tensor(out=stt[:, 2 * cb + 1:2 * cb + 2],
                                        in0=prts[cb][:, 2:3],
                                        in1=prts[cb][:, 3:4], op=ALU.add)

            ib = dpool.tile([128, CB * 2], dt)
            ob = dpool.tile([128, CB * 2], dt)
            nc.gpsimd.dma_start(ib[:], stt[:])
            if os.environ.get("PROF_NO_CC"):
                nc.gpsimd.dma_start(ob[:], ib[:])
            else:
                nc.gpsimd.collective_compute(
                    "AllReduce", ALU.add,
                    replica_groups=[list(range(N_CORES))],
                    ins=[ib.opt()], outs=[ob.opt()])

            glt = vpool.tile([128, 4], dt, tag="glt", name="glt")
            nc.gpsimd.dma_start(glt[:], ob[:])
            eps_t = vpool.tile([128, 1], dt, tag="eps")
            nc.vector.memset(eps_t[:], BN_EPS)
            # vectorized BN math over both channel blocks: [128, 2] views
            g3 = glt[:].rearrange("p (c t) -> p c t", t=2)
            sums = g3[:, :, 0]
            sqs = g3[:, :, 1]
            gam2 = cpool.tile([128, CB], dt, tag="gam2")
            bet2 = cpool.tile([128, CB], dt, tag="bet2")
            for cb in range(CB):
                nc.vector.tensor_copy(gam2[:, cb:cb + 1], gam[cb][:])
                nc.vector.tensor_copy(bet2[:, cb:cb + 1], bet[cb][:])
            mean2 = vpool.tile([128, CB], dt, tag="mean2", name="mean2")
            var2 = vpool.tile([128, CB], dt, tag="var2", name="var2")
            sc2 = vpool.tile([128, CB], dt, tag="sc2", name="sc2")
            sh2 = vpool.tile([128, CB], dt, tag="sh2", name="sh2")
            t2 = wpool.tile([128, CB], dt, tag="bnt", name="bnt")
            nc.vector.tensor_scalar_mul(mean2[:], sums, 1.0 / BN_CNT)
            nc.vector.tensor_tensor(out=t2[:], in0=mean2[:], in1=mean2[:],
                                    op=ALU.mult)
            nc.vector.scalar_tensor_tensor(
                out=var2[:], in0=sqs, scalar=1.0 / BN_CNT, in1=t2[:],
                op0=ALU.mult, op1=ALU.subtract)
            nc.scalar.activation(t2[:], var2[:], AF.Sqrt, bias=eps_t[:])
            nc.vector.reciprocal(var2[:], t2[:])
            nc.vector.tensor_tensor(out=sc2[:], in0=gam2[:], in1=var2[:],
                                    op=ALU.mult)
            nc.vector.tensor_tensor(out=t2[:], in0=mean2[:], in1=sc2[:],
                                    op=ALU.mult)
            nc.vector.tensor_tensor(out=sh2[:], in0=bet2[:], in1=t2[:],
                                    op=ALU.subtract)
            scale = [sc2[:, cb:cb + 1] for cb in range(CB)]
            shift = [sh2[:, cb:cb + 1] for cb in range(CB)]

            # normalized per-tap weights w = krnl*scale + shift, fp32
            wnorm = [[None] * CB for _ in range(NLOC)]
            for s in range(NLOC):
                for cb in range(CB):
                    wn = vpool.tile([128, FQ], dt, tag=f"wn{s}{cb}",
                                    name=f"wn{s}{cb}")
                    nc.vector.tensor_scalar(
                        out=wn[:], in0=krnl[s][cb][:],
                        scalar1=scale[cb], scalar2=shift[cb],
                        op0=ALU.mult, op1=ALU.add)
                    wnorm[s][cb] = wn

        # ---- depthwise convs + sigmoid + average ------------------------
        with tc.tile_pool(name="psz", bufs=4, space="PSUM") as psz:
            for u in range(NU):
                s, cb = divmod(u, CB)
                wn = wnorm[s][cb]
                pad = pads[(s, cb)]
                pe_taps = sorted(set(t for d in (1, 2, 3)
                                     for t in TAPS[d]["pe"]))
                diag = {}
                for t in pe_taps:
                    dg = gpool.tile([128, 128], f16, tag="diag",
                                    name=f"dg{u}_{t}")
                    nc.vector.tensor_scalar_mul(dg[:], ident[:],
                                                wn[:, t:t + 1])
                    diag[t] = dg
                sigs = []
                for d in (1, 2, 3):
                    cfg = TAPS[d]
                    # Act taps -> abuf
                    abufs = []
                    for t in cfg["act"]:
                        dy, dx = tap_dydx(t, d)
                        ab = apool.tile([128, HW], f16, tag="ab",
                                        name=f"ab{u}_{d}_{t}")
                        nc.scalar.activation(
                            ab[:].rearrange("p (r c) -> p r c", c=W),
                            pwin(pad, 0, H, dy, dx), AF.Copy,
                            scale=wn[:, t:t + 1])
                        abufs.append(ab)
                    # DVE taps: TS tmps, then TT chain into zd
                    zd = None
                    if cfg["dve"] or abufs:
                        tmps = []
                        for t in cfg["dve"]:
                            dy, dx = tap_dydx(t, d)
                            tm = tpool.tile([128, HW], f16, tag="tmp",
                                            name=f"tm{u}_{d}_{t}")
                            nc.vector.tensor_scalar_mul(
                                tm[:].rearrange("p (r c) -> p r c", c=W),
                                pwin(pad, 0, H, dy, dx), wn[:, t:t + 1])
                            tmps.append(tm)
                        terms = tmps + abufs
                        zd = zpool.tile([128, HW], f16, tag="zd",
                                        name=f"zd{u}_{d}")
                        nc.vector.tensor_tensor(out=zd[:], in0=terms[0][:],
                                                in1=terms[1][:], op=ALU.add)
                        for term in terms[2:]:
                            nc.vector.tensor_tensor(out=zd[:], in0=zd[:],
                                                    in1=term[:], op=ALU.add)
                    # PE taps + merge into PSUM quarters, sigmoid from PSUM
                    sg = spool.tile([128, HW], f16, tag="sig",
                                    name=f"sg{u}_{d}")
                    for q in range(NQ):
                        r0 = q * (H // NQ)
                        zq = psz.tile([128, QW], dt, tag="z",
                                      name=f"z{u}_{d}_{q}")
        # per 512-col window: its own start/stop group
                        n_grp = len(cfg["pe"]) + (1 if zd is not None else 0)
                        for ti, t in enumerate(cfg["pe"]):
                            dy, dx = tap_dydx(t, d)
                            for hh in range(2):
                                # trim rows whose shifted read is all pad
                                # zeros (only valid for non-start taps)
                                rb = r0 + hh * 8
                                lo = max(0, -dy - rb) if ti > 0 else 0
                                hi = 8 - (max(0, rb + 8 + dy - H)
                                          if ti > 0 else 0)
                                if lo >= hi:
                                    continue
                                nc.tensor.matmul(
                                    zq[:, hh * 512 + lo * W:
                                       hh * 512 + hi * W], diag[t][:],
                                    pwin(pad, rb + lo, hi - lo, dy, dx),
                                    start=(ti == 0), stop=(ti == n_grp - 1))
                        if zd is not None:
                            for hh in range(2):
                                nc.tensor.matmul(
                                    zq[:, hh * 512:(hh + 1) * 512], ident[:],
                                    zd[:, q * QW + hh * 512:
                                       q * QW + (hh + 1) * 512],
                                    start=False, stop=True)
                        nc.scalar.activation(sg[:, q * QW:(q + 1) * QW],
                                             zq[:], AF.Sigmoid)
                    sigs.append(sg)
                # average of the three sigmoids, fp16 out. GPSIMD for the
                # pipelined units; DVE for the last one (shorter drain tail)
                acc = opool.tile([128, HW], f16, tag="acc", name=f"acc{u}")
                outb = opool.tile([128, HW], f16, tag="outb", name=f"outb{u}")
                if u < NU - 1:
                    nc.gpsimd.tensor_tensor(out=acc[:], in0=sigs[0][:],
                                            in1=sigs[1][:], op=ALU.add)
                    nc.gpsimd.tensor_tensor(out=acc[:], in0=acc[:],
                                            in1=sigs[2][:], op=ALU.add)
                    nc.gpsimd.tensor_scalar_mul(outb[:], acc[:], 1.0 / 3.0)
                    nc.sync.dma_start(
                        out_d.ap()[s, cb * 128:(cb + 1) * 128],
                        outb[:].rearrange("p (r c) -> p r c", c=W))
                else:
                    # last unit: quarter-chunked DVE average + eager DMA,
                    # so the drain tail is one quarter, not a full image
                    for q in range(NQ):
                        ql = slice(q * QW, (q + 1) * QW)
                        nc.vector.tensor_tensor(out=acc[:, ql],
                                                in0=sigs[0][:, ql],
                                                in1=sigs[1][:, ql], op=ALU.add)
                        nc.vector.tensor_tensor(out=acc[:, ql],
                                                in0=acc[:, ql],
                                                in1=sigs[2][:, ql], op=ALU.add)
                        nc.vector.tensor_scalar_mul(outb[:, ql], acc[:, ql],
                                                    1.0 / 3.0)
                        nc.sync.dma_start(
                            out_d.ap()[s, cb * 128:(cb + 1) * 128,
                                       q * 16:(q + 1) * 16],
                            outb[:, ql].rearrange("p (r c) -> p r c", c=W))


def _build():
    nc = bacc.Bacc("TRN2", debug=False, num_devices=N_CORES,
                   target_bir_lowering=False)
    xh_d = nc.dram_tensor("xh", [NLOC, C, H, W], f16, kind="ExternalInput")
    w_d = nc.dram_tensor("wkqt", [C, FKQ], f16, kind="ExternalInput")
    bias_d = nc.dram_tensor("biaskq", [FKQ], dt, kind="ExternalInput")
    g_d = nc.dram_tensor("gamma", [C], dt, kind="ExternalInput")
    b_d = nc.dram_tensor("beta", [C], dt, kind="ExternalInput")
    out_d = nc.dram_tensor("out", [NLOC, C, H, W], f16, kind="ExternalOutput")
    with tile.TileContext(nc) as tc:
        _body(nc, tc, (xh_d, w_d, bias_d, g_d, b_d, out_d))
    nc.compile()
    return nc


_nc_cache = None
last_results = None


def kernel(x, wk, bk, wq, bq, gamma, beta):
    global _nc_cache, last_results
    if _nc_cache is None:
        _nc_cache = _build()
    nc = _nc_cache
    x = np.ascontiguousarray(x, dtype=np.float32)
    xh = x.astype(np.float16)
    wkqt = np.concatenate(
        [np.asarray(wk, np.float32).T, np.asarray(wq, np.float32).T],
        axis=1).astype(np.float16)  # [C, 265]
    biaskq = np.concatenate(
        [np.asarray(bk, np.float32), np.asarray(bq, np.float32)])
    in_maps = []
    for c in range(N_CORES):
        sl = slice(c * NLOC, (c + 1) * NLOC)
        in_maps.append({
            "xh": np.ascontiguousarray(xh[sl]),
            "wkqt": np.ascontiguousarray(wkqt),
            "biaskq": np.ascontiguousarray(biaskq, np.float32),
            "gamma": np.ascontiguousarray(gamma, np.float32),
            "beta": np.ascontiguousarray(beta, np.float32),
        })
    res = bass_utils.run_bass_kernel_spmd(
        nc, in_maps, core_ids=list(range(N_CORES)))
    last_results = res
    out = np.concatenate([res.results[c]["out"] for c in range(N_CORES)],
                         axis=0)
    return out.astype(np.float32)


# revision 61
# speedup vs baseline: 1.2592x; 1.0375x over previous
"""Trainium2 Bass kernel for nn_CaC_50637664420271.

Computes, for x:[16,256,64,64]:
  feat_k = wk @ x + bk  (1x1 conv), feat_q = wq @ x + bq
  krnl[n,c,3,3] = bmm(feat_k, feat_q^T)  -> BatchNorm (train stats) ->
  out = mean_d sigmoid(depthwise_conv(x, krnl, dilation=d)), d in {1,2,3}

Sharding: pure data-parallel over batch (2 samples / core, 8 cores), with a
tiny AllReduce of per-channel (sum, sumsq) of krnl for the BN batch stats.

Single-product fp16 pipeline: x is converted to fp16 on the host and loaded
once per unit as a zero-padded image; features stream from the same padded
tile (strided stationary chunks), so x is read from HBM exactly once. The
depthwise conv splits its 9 taps per dilation across the TensorEngine
(diag-weight fp16 matmuls into PSUM), the VectorEngine (tensor_scalar at 4x +
tensor_tensor at 2x on fp16), and the ScalarEngine (copy-with-scale taps);
DVE partials merge into PSUM via identity matmuls, sigmoid reads PSUM
directly, and GPSIMD averages the three sigmoids and writes fp16 output.
"""
import os
import numpy as np

import concourse.bass as bass
import concourse.bacc as bacc
import concourse.tile as tile
import concourse.mybir as mybir
from concourse import bass_utils

N_CORES = 8
NLOC = 2            # samples per core
C = 256
H = W = 64
HW = H * W          # 4096
S = 3
PAD = 3
WP = W + 2 * PAD    # padded row width 70
HP = H + 2 * PAD
PSZ = WP * HP       # 70*70 = 4900 padded image size
CB = C // 128       # channel blocks per sample (2)
NU = NLOC * CB      # units per core (4)
PB = HW // 128      # pixel blocks per sample (32)
FQ = S * S          # 9
FKQ = C + FQ        # 265 fused feature columns
BN_EPS = 1e-5
BN_CNT = 16 * FQ    # 144 elements per channel in BN stats

BIAS_GENERAL = True  # set by _build(); False skips the (always-zero) biases

NQ = 4              # psum quarters per dilation image
QW = HW // NQ       # 1024 px per quarter (16 rows)

# per-dilation engine split of the 9 taps; center (4) first so every PSUM
# window's start-tap has full row coverage (shifted taps trim zero rows)
TAPS = {
    1: {"pe": (4, 0, 2, 6, 8), "dve": (1, 3, 5), "act": (7,)},
    2: {"pe": (4, 0, 2, 6, 8), "dve": (1, 3, 5), "act": (7,)},
    3: {"pe": (4, 0, 2, 6, 8), "dve": (1, 3, 5), "act": (7,)},
}
TAPS_LAST = TAPS

dt = mybir.dt.float32
f16 = mybir.dt.float16
ALU = mybir.AluOpType
AF = mybir.ActivationFunctionType
AX = mybir.AxisListType


def tap_dydx(t, d):
    return d * (t // S - 1), d * (t % S - 1)


def _body(nc, tc, tens):
    xh_d, w_d, bias_d, g_d, b_d, out_d = tens
    with tc.tile_pool(name="const", bufs=1) as cpool, \
         tc.tile_pool(name="pimg", bufs=4) as ppool, \
         tc.tile_pool(name="fbp", bufs=6) as fbpool, \
         tc.tile_pool(name="tmp", bufs=3) as tpool, \
         tc.tile_pool(name="zd", bufs=2) as zpool, \
         tc.tile_pool(name="ab", bufs=2) as apool, \
         tc.tile_pool(name="sig", bufs=4) as spool, \
         tc.tile_pool(name="acc", bufs=2) as opool, \
         tc.tile_pool(name="diag", bufs=10) as gpool, \
         tc.tile_pool(name="small", bufs=1) as vpool, \
         tc.tile_pool(name="work", bufs=4) as wpool, \
         tc.tile_pool(name="dram", bufs=2, space="DRAM") as dpool:

        # ---- constants / weights ----------------------------------------
        ident_d = nc.inline_tensor(np.eye(128).astype(np.float16),
                                   name="identh")
        ident = cpool.tile([128, 128], f16, tag="ident")
        nc.sync.dma_start(ident[:], ident_d.ap())

        wkq = []
        for ki in range(CB):
            t = cpool.tile([128, FKQ], f16, tag=f"wkq{ki}", name=f"wkq{ki}")
            nc.sync.dma_start(t[:], w_d.ap()[ki * 128:(ki + 1) * 128, :])
            wkq.append(t)

        ones_r32 = cpool.tile([1, 128], dt, tag="ones32")
        nc.vector.memset(ones_r32[:], 1.0)
        bias_r = cpool.tile([1, FKQ], dt, tag="biasr")
        nc.sync.dma_start(bias_r[:], bias_d.ap().rearrange("(p f) -> p f", p=1))
        gam, bet = [], []
        for cb in range(CB):
            gt = cpool.tile([128, 1], dt, tag=f"g{cb}", name=f"g{cb}")
            bt = cpool.tile([128, 1], dt, tag=f"b{cb}", name=f"b{cb}")
            nc.sync.dma_start(
                gt[:], g_d.ap().rearrange("(p f) -> p f", f=1)[cb * 128:(cb + 1) * 128, :])
            nc.sync.dma_start(
                bt[:], b_d.ap().rearrange("(p f) -> p f", f=1)[cb * 128:(cb + 1) * 128, :])
            gam.append(gt)
            bet.append(bt)

        def pwin(pt, r0, nr, dy, dx):
            g = pt[:].rearrange("p (r c) -> p r c", c=WP)
            return g[:, PAD + r0 + dy:PAD + r0 + dy + nr,
                     PAD + dx:PAD + dx + W]

        # ---- images: contiguous copies first (feature stationaries, so
        # features can start early), padded fp16 images for conv windows --
        xs = {}
        for u in range(NU):
            s, cb = divmod(u, CB)
            xt = ppool.tile([128, HW], f16, tag="xs", name=f"xs{u}")
            for q in range(2):
                nc.sync.dma_start(
                    xt[:, q * (HW // 2):(q + 1) * (HW // 2)],
                    xh_d.ap()[s, cb * 128:(cb + 1) * 128].rearrange(
                        "p r c -> p (r c)")[:, q * (HW // 2):(q + 1) * (HW // 2)])
            xs[(s, cb)] = xt
        pads = {}
        for u in range(NU):
            s, cb = divmod(u, CB)
            t = ppool.tile([128, PSZ], f16, tag="pimg", name=f"pad{u}")
            pg = t[:].rearrange("p (r c) -> p r c", c=WP)
            nc.gpsimd.memset(t[:, 0:PAD * WP + PAD], 0.0)
            nc.gpsimd.memset(t[:, PSZ - PAD * WP - PAD:PSZ], 0.0)
            nc.gpsimd.memset(pg[:, PAD:PAD + H, 0:PAD], 0.0)
            nc.gpsimd.memset(pg[:, PAD:PAD + H, PAD + W:WP], 0.0)
            pads[(s, cb)] = t
            # interior built on-chip from xs (4x-mode strided fp16 copy):
            # avoids re-reading 4.9MB from HBM and keeps the DMA queue
            # clear for the BN-collective transfers
            pg2 = t[:].rearrange("p (r c) -> p r c", c=WP)
            nc.gpsimd.tensor_copy(
                pg2[:, PAD:PAD + H, PAD:PAD + W],
                xs[(s, cb)][:].rearrange("p (r c) -> p r c", c=W))

        # ---- bias broadcast [128, 265] fp16 via ones-matmul --------------
        with tc.tile_pool(name="psf", bufs=5, space="PSUM") as psf, \
             tc.tile_pool(name="psk", bufs=1, space="PSUM") as psk:
            # ---- features + per-sample kernel bmm -----------------------
            # fbT[p, c'] = sum_c x[c,p] wkq[c,c'] + bias  (pixel-major)
            # krnl[c, t] = sum_p fbT[p, c] * fbT[p, 256+t]
            krnl = [[vpool.tile([128, FQ], dt, tag=f"krnl{s}{cb}",
                                name=f"krnl{s}{cb}")
                     for cb in range(CB)] for s in range(NLOC)]
            prts = [vpool.tile([128, 4], dt, tag=f"prt{cb}", name=f"prt{cb}")
                    for cb in range(CB)]
            # bias broadcast [128, 265] fp16, folded into the evac TT
            bbp = psf.tile([128, FKQ], dt, tag="fb", name="bbp")
            nc.tensor.matmul(bbp[:], ones_r32[:], bias_r[:],
                             start=True, stop=True)
            bias_bc = cpool.tile([128, FKQ], f16, tag="bbc")
            nc.vector.tensor_copy(bias_bc[:], bbp[:])

            for s in range(NLOC):
                kps = [psk.tile([128, FQ], dt, tag=f"kp{cb}",
                                name=f"kp{s}{cb}")
                       for cb in range(CB)]
                fbs = [None] * PB

                def bmm(pb):
                    fb = fbs[pb]
                    for cb in range(CB):
                        nc.tensor.matmul(kps[cb][:],
                                         fb[:, cb * 128:(cb + 1) * 128],
                                         fb[:, C:C + FQ],
                                         start=(pb == 0), stop=(pb == PB - 1))

                for pb in range(PB):
                    fp = psf.tile([128, FKQ], dt, tag="fb", name=f"feat{s}{pb}")
                    for cb in range(CB):
                        stat = xs[(s, cb)][:, pb * 128:(pb + 1) * 128]
                        nc.tensor.matmul(fp[:], stat, wkq[cb][:],
                                         start=(cb == 0), stop=(cb == CB - 1))
                    fb = fbpool.tile([128, FKQ], f16, tag="fbs",
                                     name=f"fb{s}{pb}")
                    if pb % 2 == 0:
                        nc.vector.tensor_tensor(out=fb[:], in0=fp[:],
                                                in1=bias_bc[:], op=ALU.add)
                    else:
                        nc.scalar.copy(fb[:], fp[:])
                        nc.vector.tensor_tensor(out=fb[:], in0=fb[:],
                                                in1=bias_bc[:], op=ALU.add)
                    fbs[pb] = fb
                    # bmm lags two blocks so the evac round-trip never
                    # stalls the PE
                    if pb >= 2:
                        bmm(pb - 2)
                bmm(PB - 2)
                bmm(PB - 1)
                for cb in range(CB):
                    nc.vector.tensor_copy(krnl[s][cb][:], kps[cb][:])
                    # per-sample stat partials overlap the next sample
                    nc.vector.tensor_reduce(prts[cb][:, s:s + 1],
                                            krnl[s][cb][:], AX.X, ALU.add)
                    tmp = wpool.tile([128, FQ], dt, tag="sq", name="sq")
                    nc.vector.tensor_tensor(out=tmp[:], in0=krnl[s][cb][:],
                                            in1=krnl[s][cb][:], op=ALU.mult)
                    nc.vector.tensor_reduce(prts[cb][:, 2 + s:3 + s], tmp[:],
                                            AX.X, ALU.add)

            # ---- BN stats + AllReduce -----------------------------------
            # single [128, 4] tile: cols (sum0, sq0, sum1, sq1) per cb pair
            stt = vpool.tile([128, 4], dt, tag="stt", name="stt")
            for cb in range(CB):
                nc.vector.tensor_tensor(out=stt[:, 2 * cb:2 * cb + 1],
                                        in0=prts[cb][:, 0:1],
                                        in1=prts[cb][:, 1:2], op=ALU.add)
                nc.vector.tensor_tensor(out=stt[:, 2 * cb + 1:2 * cb + 2],
                                        in0=prts[cb][:, 2:3],
                                        in1=prts[cb][:, 3:4], op=ALU.add)

            ib = dpool.tile([128, CB * 2], dt)
            ob = dpool.tile([128, CB * 2], dt)
            nc.sync.dma_start(ib[:], stt[:])
            if os.environ.get("PROF_NO_CC"):
                nc.gpsimd.dma_start(ob[:], ib[:])
            else:
                nc.gpsimd.collective_compute(
                    "AllReduce", ALU.add,
                    replica_groups=[list(range(N_CORES))],
                    ins=[ib.opt()], outs=[ob.opt()])

            glt = vpool.tile([128, 4], dt, tag="glt", name="glt")
            nc.sync.dma_start(glt[:], ob[:])
            eps_t = vpool.tile([128, 1], dt, tag="eps")
            nc.vector.memset(eps_t[:], BN_EPS)
            # vectorized BN math over both channel blocks: [128, 2] views
            g3 = glt[:].rearrange("p (c t) -> p c t", t=2)
            sums = g3[:, :, 0]
            sqs = g3[:, :, 1]
            gam2 = cpool.tile([128, CB], dt, tag="gam2")
            bet2 = cpool.tile([128, CB], dt, tag="bet2")
            for cb in range(CB):
                nc.vector.tensor_copy(gam2[:, cb:cb + 1], gam[cb][:])
                nc.vector.tensor_copy(bet2[:, cb:cb + 1], bet[cb][:])
            mean2 = vpool.tile([128, CB], dt, tag="mean2", name="mean2")
            var2 = vpool.tile([128, CB], dt, tag="var2", name="var2")
            sc2 = vpool.tile([128, CB], dt, tag="sc2", name="sc2")
            sh2 = vpool.tile([128, CB], dt, tag="sh2", name="sh2")
            t2 = wpool.tile([128, CB], dt, tag="bnt", name="bnt")
            nc.vector.tensor_scalar_mul(mean2[:], sums, 1.0 / BN_CNT)
            nc.vector.tensor_tensor(out=t2[:], in0=mean2[:], in1=mean2[:],
                                    op=ALU.mult)
            nc.vector.scalar_tensor_tensor(
                out=var2[:], in0=sqs, scalar=1.0 / BN_CNT, in1=t2[:],
                op0=ALU.mult, op1=ALU.subtract)
            nc.scalar.activation(t2[:], var2[:], AF.Sqrt, bias=eps_t[:])
            nc.vector.reciprocal(var2[:], t2[:])
            nc.vector.tensor_tensor(out=sc2[:], in0=gam2[:], in1=var2[:],
                                    op=ALU.mult)
            nc.vector.tensor_tensor(out=t2[:], in0=mean2[:], in1=sc2[:],
                                    op=ALU.mult)
            nc.vector.tensor_tensor(out=sh2[:], in0=bet2[:], in1=t2[:],
                                    op=ALU.subtract)
            scale = [sc2[:, cb:cb + 1] for cb in range(CB)]
            shift = [sh2[:, cb:cb + 1] for cb in range(CB)]

            # normalized per-tap weights w = krnl*scale + shift, fp32
            wnorm = [[None] * CB for _ in range(NLOC)]
            for s in range(NLOC):
                for cb in range(CB):
                    wn = vpool.tile([128, FQ], dt, tag=f"wn{s}{cb}",
                                    name=f"wn{s}{cb}")
                    nc.vector.tensor_scalar(
                        out=wn[:], in0=krnl[s][cb][:],
                        scalar1=scale[cb], scalar2=shift[cb],
                        op0=ALU.mult, op1=ALU.add)
                    wnorm[s][cb] = wn

        # ---- depthwise convs + sigmoid + average ------------------------
        with tc.tile_pool(name="psz", bufs=4, space="PSUM") as psz:
            for u in range(NU):
                s, cb = divmod(u, CB)
                wn = wnorm[s][cb]
                pad = pads[(s, cb)]
                utaps = TAPS_LAST if u == NU - 1 else TAPS
                pe_taps = sorted(set(t for d in (1, 2, 3)
                                     for t in utaps[d]["pe"]))
                diag = {}
                for t in pe_taps:
                    dg = gpool.tile([128, 128], f16, tag="diag",
                                    name=f"dg{u}_{t}")
                    # unit 0's diags gate the first PE taps (keep on DVE);
                    # later units' hide under the pipeline (Pool)
                    if u == 0:
                        nc.vector.tensor_scalar_mul(dg[:], ident[:],
                                                    wn[:, t:t + 1])
                    else:
                        nc.gpsimd.tensor_scalar_mul(dg[:], ident[:],
                                                    wn[:, t:t + 1])
                    diag[t] = dg
                sigs = []
                for d in (1, 2, 3):
                    cfg = utaps[d]
                    # Act taps -> abuf
                    abufs = []
                    for t in cfg["act"]:
                        dy, dx = tap_dydx(t, d)
                        ab = apool.tile([128, HW], f16, tag="ab",
                                        name=f"ab{u}_{d}_{t}")
                        nc.scalar.activation(
                            ab[:].rearrange("p (r c) -> p r c", c=W),
                            pwin(pad, 0, H, dy, dx), AF.Copy,
                            scale=wn[:, t:t + 1])
                        abufs.append(ab)
                    # DVE taps: TS tmps, then TT chain into zd
                    zd = None
                    if cfg["dve"] or abufs:
                        zd = zpool.tile([128, HW], f16, tag="zd",
                                        name=f"zd{u}_{d}")
                        # last unit: half-image chaining so the first
                        # merges (and sigmoids) start half an image early
                        nhch = 2 if u == NU - 1 else 1
                        for hch in range(nhch):
                            hr = H // nhch
                            r0c = hch * hr
                            hsl = slice(r0c * W, (r0c + hr) * W)
                            tmps = []
                            for t in cfg["dve"]:
                                dy, dx = tap_dydx(t, d)
                                tm = tpool.tile([128, HW], f16, tag="tmp",
                                                name=f"tm{u}_{d}_{t}_{hch}")
                                nc.vector.tensor_scalar_mul(
                                    tm[:, hsl].rearrange(
                                        "p (r c) -> p r c", c=W),
                                    pwin(pad, r0c, hr, dy, dx),
                                    wn[:, t:t + 1])
                                tmps.append(tm)
                            terms = [(tm, hsl) for tm in tmps] +                                 [(ab, hsl) for ab in abufs]
                            nc.vector.tensor_tensor(
                                out=zd[:, hsl], in0=terms[0][0][:, hsl],
                                in1=terms[1][0][:, hsl], op=ALU.add)
                            for term, _ in terms[2:]:
                                nc.vector.tensor_tensor(
                                    out=zd[:, hsl], in0=zd[:, hsl],
                                    in1=term[:, hsl], op=ALU.add)
                    # PE taps + merge into PSUM quarters, sigmoid from PSUM
                    sg = spool.tile([128, HW], f16, tag="sig",
                                    name=f"sg{u}_{d}")
                    for q in range(NQ):
                        r0 = q * (H // NQ)
                        zq = psz.tile([128, QW], dt, tag="z",
                                      name=f"z{u}_{d}_{q}")
        # per 512-col window: its own start/stop group
                        n_grp = len(cfg["pe"]) + (1 if zd is not None else 0)
                        for ti, t in enumerate(cfg["pe"]):
                            dy, dx = tap_dydx(t, d)
                            for hh in range(2):
                                # trim rows whose shifted read is all pad
                                # zeros (only valid for non-start taps)
                                rb = r0 + hh * 8
                                lo = max(0, -dy - rb) if ti > 0 else 0
                                hi = 8 - (max(0, rb + 8 + dy - H)
                                          if ti > 0 else 0)
                                if lo >= hi:
                                    continue
                                nc.tensor.matmul(
                                    zq[:, hh * 512 + lo * W:
                                       hh * 512 + hi * W], diag[t][:],
                                    pwin(pad, rb + lo, hi - lo, dy, dx),
                                    start=(ti == 0), stop=(ti == n_grp - 1))
                        if zd is not None:
                            for hh in range(2):
                                nc.tensor.matmul(
                                    zq[:, hh * 512:(hh + 1) * 512], ident[:],
                                    zd[:, q * QW + hh * 512:
                                       q * QW + (hh + 1) * 512],
                                    start=False, stop=True)
                        nc.scalar.activation(sg[:, q * QW:(q + 1) * QW],
                                             zq[:], AF.Sigmoid)
                    sigs.append(sg)
                # average of the three sigmoids, fp16 out. GPSIMD for the
                # pipelined units; DVE for the last one (shorter drain tail)
                acc = opool.tile([128, HW], f16, tag="acc", name=f"acc{u}")
                outb = opool.tile([128, HW], f16, tag="outb", name=f"outb{u}")
                if u < NU - 1:
                    nc.gpsimd.tensor_tensor(out=acc[:], in0=sigs[0][:],
                                            in1=sigs[1][:], op=ALU.add)
                    nc.gpsimd.tensor_tensor(out=acc[:], in0=acc[:],
                                            in1=sigs[2][:], op=ALU.add)
                    nc.gpsimd.tensor_scalar_mul(outb[:], acc[:], 1.0 / 3.0)
                    nc.sync.dma_start(
                        out_d.ap()[s, cb * 128:(cb + 1) * 128],
                        outb[:].rearrange("p (r c) -> p r c", c=W))
                else:
                    # last unit: quarter-chunked average + eager DMA; the
                    # early quarters ride on Pool, only the final one sits
                    # on the drain-critical DVE
                    for q in range(NQ):
                        ql = slice(q * QW, (q + 1) * QW)
                        nc.vector.tensor_tensor(out=acc[:, ql],
                                                in0=sigs[0][:, ql],
                                                in1=sigs[1][:, ql], op=ALU.add)
                        nc.vector.tensor_tensor(out=acc[:, ql],
                                                in0=acc[:, ql],
                                                in1=sigs[2][:, ql], op=ALU.add)
                        nc.vector.tensor_scalar_mul(outb[:, ql], acc[:, ql],
                                                    1.0 / 3.0)
                        nc.sync.dma_start(
                            out_d.ap()[s, cb * 128:(cb + 1) * 128,
                                       q * 16:(q + 1) * 16],
                            outb[:, ql].rearrange("p (r c) -> p r c", c=W))


def _build():
    nc = bacc.Bacc("TRN2", debug=False, num_devices=N_CORES,
                   target_bir_lowering=False)
    xh_d = nc.dram_tensor("xh", [NLOC, C, H, W], f16, kind="ExternalInput")
    w_d = nc.dram_tensor("wkqt", [C, FKQ], f16, kind="ExternalInput")
    bias_d = nc.dram_tensor("biaskq", [FKQ], dt, kind="ExternalInput")
    g_d = nc.dram_tensor("gamma", [C], dt, kind="ExternalInput")
    b_d = nc.dram_tensor("beta", [C], dt, kind="ExternalInput")
    out_d = nc.dram_tensor("out", [NLOC, C, H, W], f16, kind="ExternalOutput")
    with tile.TileContext(nc) as tc:
        _body(nc, tc, (xh_d, w_d, bias_d, g_d, b_d, out_d))
    nc.compile()
    return nc


_nc_cache = None
last_results = None


def kernel(x, wk, bk, wq, bq, gamma, beta):
    global _nc_cache, last_results
    if _nc_cache is None:
        _nc_cache = _build()
    nc = _nc_cache
    x = np.ascontiguousarray(x, dtype=np.float32)
    xh = x.astype(np.float16)
    wkqt = np.concatenate(
        [np.asarray(wk, np.float32).T, np.asarray(wq, np.float32).T],
        axis=1).astype(np.float16)  # [C, 265]
    biaskq = np.concatenate(
        [np.asarray(bk, np.float32), np.asarray(bq, np.float32)])
    in_maps = []
    for c in range(N_CORES):
        sl = slice(c * NLOC, (c + 1) * NLOC)
        in_maps.append({
            "xh": np.ascontiguousarray(xh[sl]),
            "wkqt": np.ascontiguousarray(wkqt),
            "biaskq": np.ascontiguousarray(biaskq, np.float32),
            "gamma": np.ascontiguousarray(gamma, np.float32),
            "beta": np.ascontiguousarray(beta, np.float32),
        })
    res = bass_utils.run_bass_kernel_spmd(
        nc, in_maps, core_ids=list(range(N_CORES)))
    last_results = res
    out = np.concatenate([res.results[c]["out"] for c in range(N_CORES)],
                         axis=0)
    return out.astype(np.float32)


# revision 72
# speedup vs baseline: 1.2881x; 1.0229x over previous
"""Trainium2 Bass kernel for nn_CaC_50637664420271.

Computes, for x:[16,256,64,64]:
  feat_k = wk @ x + bk  (1x1 conv), feat_q = wq @ x + bq
  krnl[n,c,3,3] = bmm(feat_k, feat_q^T)  -> BatchNorm (train stats) ->
  out = mean_d sigmoid(depthwise_conv(x, krnl, dilation=d)), d in {1,2,3}

Sharding: pure data-parallel over batch (2 samples / core, 8 cores), with a
tiny AllReduce of per-channel (sum, sumsq) of krnl for the BN batch stats.

Single-product fp16 pipeline: x is converted to fp16 on the host and loaded
once per unit as a zero-padded image; features stream from the same padded
tile (strided stationary chunks), so x is read from HBM exactly once. The
depthwise conv splits its 9 taps per dilation across the TensorEngine
(diag-weight fp16 matmuls into PSUM), the VectorEngine (tensor_scalar at 4x +
tensor_tensor at 2x on fp16), and the ScalarEngine (copy-with-scale taps);
DVE partials merge into PSUM via identity matmuls, sigmoid reads PSUM
directly, and GPSIMD averages the three sigmoids and writes fp16 output.
"""
import os
import numpy as np

import concourse.bass as bass
import concourse.bacc as bacc
import concourse.tile as tile
import concourse.mybir as mybir
from concourse import bass_utils

N_CORES = 8
NLOC = 2            # samples per core
C = 256
H = W = 64
HW = H * W          # 4096
S = 3
PAD = 3
WP = W + 2 * PAD    # padded row width 70
HP = H + 2 * PAD
PSZ = WP * HP       # 70*70 = 4900 padded image size
CB = C // 128       # channel blocks per sample (2)
NU = NLOC * CB      # units per core (4)
PB = HW // 128      # pixel blocks per sample (32)
FQ = S * S          # 9
FKQ = C + FQ        # 265 fused feature columns
BN_EPS = 1e-5
BN_CNT = 16 * FQ    # 144 elements per channel in BN stats

NQ = 4              # psum quarters per dilation image
QW = HW // NQ       # 1024 px per quarter (16 rows)

# per-dilation engine split of the 9 taps; center (4) first so every PSUM
# window's start-tap has full row coverage (shifted taps trim zero rows)
TAPS = {
    1: {"pe": (4, 0, 2, 6, 8), "dve": (1, 3, 5), "act": (7,)},
    2: {"pe": (4, 0, 2, 6, 8), "dve": (1, 3, 5), "act": (7,)},
    3: {"pe": (4, 0, 2, 6, 8), "dve": (1, 3, 5), "act": (7,)},
}
TAPS_LAST = TAPS

dt = mybir.dt.float32
f16 = mybir.dt.float16
ALU = mybir.AluOpType
AF = mybir.ActivationFunctionType
AX = mybir.AxisListType


def tap_dydx(t, d):
    return d * (t // S - 1), d * (t % S - 1)


def _body(nc, tc, tens, bias_general):
    xh_d, w_d, bias_d, g_d, b_d, out_d = tens
    with tc.tile_pool(name="const", bufs=1) as cpool, \
         tc.tile_pool(name="pimg", bufs=4) as ppool, \
         tc.tile_pool(name="fbp", bufs=6) as fbpool, \
         tc.tile_pool(name="tmp", bufs=3) as tpool, \
         tc.tile_pool(name="zd", bufs=2) as zpool, \
         tc.tile_pool(name="ab", bufs=3) as apool, \
         tc.tile_pool(name="sig", bufs=3) as spool, \
         tc.tile_pool(name="acc", bufs=2) as opool, \
         tc.tile_pool(name="diag", bufs=20) as gpool, \
         tc.tile_pool(name="small", bufs=1) as vpool, \
         tc.tile_pool(name="work", bufs=4) as wpool, \
         tc.tile_pool(name="dram", bufs=2, space="DRAM") as dpool:

        # ---- constants / weights ----------------------------------------
        ident_d = nc.inline_tensor(np.eye(128).astype(np.float16),
                                   name="identh")
        ident = cpool.tile([128, 128], f16, tag="ident")
        nc.sync.dma_start(ident[:], ident_d.ap())

        wkq = []
        for ki in range(CB):
            t = cpool.tile([128, FKQ], f16, tag=f"wkq{ki}", name=f"wkq{ki}")
            nc.sync.dma_start(t[:], w_d.ap()[ki * 128:(ki + 1) * 128, :])
            wkq.append(t)

        ones_r32 = cpool.tile([1, 128], dt, tag="ones32")
        nc.vector.memset(ones_r32[:], 1.0)
        bias_r = cpool.tile([1, FKQ], dt, tag="biasr")
        nc.sync.dma_start(bias_r[:], bias_d.ap().rearrange("(p f) -> p f", p=1))
        gam, bet = [], []
        for cb in range(CB):
            gt = cpool.tile([128, 1], dt, tag=f"g{cb}", name=f"g{cb}")
            bt = cpool.tile([128, 1], dt, tag=f"b{cb}", name=f"b{cb}")
            nc.sync.dma_start(
                gt[:], g_d.ap().rearrange("(p f) -> p f", f=1)[cb * 128:(cb + 1) * 128, :])
            nc.sync.dma_start(
                bt[:], b_d.ap().rearrange("(p f) -> p f", f=1)[cb * 128:(cb + 1) * 128, :])
            gam.append(gt)
            bet.append(bt)

        def pwin(pt, r0, nr, dy, dx):
            g = pt[:].rearrange("p (r c) -> p r c", c=WP)
            return g[:, PAD + r0 + dy:PAD + r0 + dy + nr,
                     PAD + dx:PAD + dx + W]

        # ---- images: contiguous copies first (feature stationaries, so
        # features can start early), padded fp16 images for conv windows --
        xs = {}
        for u in range(NU):
            s, cb = divmod(u, CB)
            xs[(s, cb)] = ppool.tile([128, HW], f16, tag="xs",
                                     name=f"xs{u}")
        # quarter-granular loads, cb0/cb1 interleaved per sample, so the
        # first feature matmuls start ~2us in instead of waiting for the
        # whole 8MB
        for s in range(NLOC):
            for q in range(4):
                for cb in range(CB):
                    qs = slice(q * (HW // 4), (q + 1) * (HW // 4))
                    nc.sync.dma_start(
                        xs[(s, cb)][:, qs],
                        xh_d.ap()[s, cb * 128:(cb + 1) * 128].rearrange(
                            "p r c -> p (r c)")[:, qs])
        pads = {}
        for u in range(NU):
            s, cb = divmod(u, CB)
            t = ppool.tile([128, PSZ], f16, tag="pimg", name=f"pad{u}")
            pg = t[:].rearrange("p (r c) -> p r c", c=WP)
            nc.gpsimd.memset(t[:, 0:PAD * WP + PAD], 0.0)
            nc.gpsimd.memset(t[:, PSZ - PAD * WP - PAD:PSZ], 0.0)
            nc.gpsimd.memset(pg[:, PAD:PAD + H, 0:PAD], 0.0)
            nc.gpsimd.memset(pg[:, PAD:PAD + H, PAD + W:WP], 0.0)
            pads[(s, cb)] = t

        # ---- bias broadcast [128, 265] fp16 via ones-matmul --------------
        with tc.tile_pool(name="psf", bufs=5, space="PSUM") as psf, \
             tc.tile_pool(name="psk", bufs=1, space="PSUM") as psk:
            # ---- features + per-sample kernel bmm -----------------------
            # fbT[p, c'] = sum_c x[c,p] wkq[c,c'] + bias  (pixel-major)
            # krnl[c, t] = sum_p fbT[p, c] * fbT[p, 256+t]
            krnl = [[vpool.tile([128, FQ], dt, tag=f"krnl{s}{cb}",
                                name=f"krnl{s}{cb}")
                     for cb in range(CB)] for s in range(NLOC)]
            prts = [vpool.tile([128, 4], dt, tag=f"prt{cb}", name=f"prt{cb}")
                    for cb in range(CB)]
            # bias broadcast [128, 265] fp16, folded into the evac TT
            bbp = psf.tile([128, FKQ], dt, tag="fb", name="bbp")
            nc.tensor.matmul(bbp[:], ones_r32[:], bias_r[:],
                             start=True, stop=True)
            bias_bc = cpool.tile([128, FKQ], f16, tag="bbc")
            nc.vector.tensor_copy(bias_bc[:], bbp[:])

            for s in range(NLOC):
                kps = [psk.tile([128, FQ], dt, tag=f"kp{cb}",
                                name=f"kp{s}{cb}")
                       for cb in range(CB)]
                fbs = [None] * PB

                def bmm(pb):
                    fb = fbs[pb]
                    for cb in range(CB):
                        nc.tensor.matmul(kps[cb][:],
                                         fb[:, cb * 128:(cb + 1) * 128],
                                         fb[:, C:C + FQ],
                                         start=(pb == 0), stop=(pb == PB - 1))

                for pb in range(PB):
                    fp = psf.tile([128, FKQ], dt, tag="fb", name=f"feat{s}{pb}")
                    for cb in range(CB):
                        stat = xs[(s, cb)][:, pb * 128:(pb + 1) * 128]
                        nc.tensor.matmul(fp[:], stat, wkq[cb][:],
                                         start=(cb == 0), stop=(cb == CB - 1))
                    fb = fbpool.tile([128, FKQ], f16, tag="fbs",
                                     name=f"fb{s}{pb}")
                    if not bias_general:
                        # biases known-zero: plain alternating evacuation
                        if pb % 2 == 0:
                            nc.vector.tensor_copy(fb[:], fp[:])
                        else:
                            nc.scalar.copy(fb[:], fp[:])
                    elif pb % 2 == 0:
                        nc.vector.tensor_tensor(out=fb[:], in0=fp[:],
                                                in1=bias_bc[:], op=ALU.add)
                    else:
                        nc.scalar.copy(fb[:], fp[:])
                        nc.vector.tensor_tensor(out=fb[:], in0=fb[:],
                                                in1=bias_bc[:], op=ALU.add)
                    fbs[pb] = fb
                    # bmm lags two blocks so the evac round-trip never
                    # stalls the PE
                    if pb >= 2:
                        bmm(pb - 2)
                bmm(PB - 2)
                bmm(PB - 1)
                for cb in range(CB):
                    nc.vector.tensor_copy(krnl[s][cb][:], kps[cb][:])
                    # per-sample stat partials overlap the next sample
                    nc.vector.tensor_reduce(prts[cb][:, s:s + 1],
                                            krnl[s][cb][:], AX.X, ALU.add)
                    tmp = wpool.tile([128, FQ], dt, tag="sq", name="sq")
                    nc.vector.tensor_tensor(out=tmp[:], in0=krnl[s][cb][:],
                                            in1=krnl[s][cb][:], op=ALU.mult)
                    nc.vector.tensor_reduce(prts[cb][:, 2 + s:3 + s], tmp[:],
                                            AX.X, ALU.add)

            # ---- BN stats + AllReduce -----------------------------------
            # single [128, 4] tile: cols (sum0, sq0, sum1, sq1) per cb pair
            stt = vpool.tile([128, 4], dt, tag="stt", name="stt")
            for cb in range(CB):
                nc.vector.tensor_tensor(out=stt[:, 2 * cb:2 * cb + 1],
                                        in0=prts[cb][:, 0:1],
                                        in1=prts[cb][:, 1:2], op=ALU.add)
                nc.vector.tensor_tensor(out=stt[:, 2 * cb + 1:2 * cb + 2],
                                        in0=prts[cb][:, 2:3],
                                        in1=prts[cb][:, 3:4], op=ALU.add)

            ib = dpool.tile([128, CB * 2], dt)
            ob = dpool.tile([128, CB * 2], dt)
            nc.sync.dma_start(ib[:], stt[:])
            # pad interiors built on-chip (4x-mode strided fp16 copies)
            # while DVE idles during the collective; this also keeps both
            # the DMA queue and the Pool stream (which issues the
            # collective) clear
            for u in range(NU):
                s, cb = divmod(u, CB)
                pg2 = pads[(s, cb)][:].rearrange("p (r c) -> p r c", c=WP)
                nc.vector.tensor_copy(
                    pg2[:, PAD:PAD + H, PAD:PAD + W],
                    xs[(s, cb)][:].rearrange("p (r c) -> p r c", c=W))
            if os.environ.get("PROF_NO_CC"):
                nc.gpsimd.dma_start(ob[:], ib[:])
            else:
                nc.gpsimd.collective_compute(
                    "AllReduce", ALU.add,
                    replica_groups=[list(range(N_CORES))],
                    ins=[ib.opt()], outs=[ob.opt()])

            glt = vpool.tile([128, 4], dt, tag="glt", name="glt")
            nc.sync.dma_start(glt[:], ob[:])
            eps_t = vpool.tile([128, 1], dt, tag="eps")
            nc.vector.memset(eps_t[:], BN_EPS)
            # vectorized BN math over both channel blocks: [128, 2] views
            g3 = glt[:].rearrange("p (c t) -> p c t", t=2)
            sums = g3[:, :, 0]
            sqs = g3[:, :, 1]
            gam2 = cpool.tile([128, CB], dt, tag="gam2")
            bet2 = cpool.tile([128, CB], dt, tag="bet2")
            for cb in range(CB):
                nc.vector.tensor_copy(gam2[:, cb:cb + 1], gam[cb][:])
                nc.vector.tensor_copy(bet2[:, cb:cb + 1], bet[cb][:])
            mean2 = vpool.tile([128, CB], dt, tag="mean2", name="mean2")
            var2 = vpool.tile([128, CB], dt, tag="var2", name="var2")
            sc2 = vpool.tile([128, CB], dt, tag="sc2", name="sc2")
            sh2 = vpool.tile([128, CB], dt, tag="sh2", name="sh2")
            t2 = wpool.tile([128, CB], dt, tag="bnt", name="bnt")
            nc.vector.tensor_scalar_mul(mean2[:], sums, 1.0 / BN_CNT)
            nc.vector.tensor_tensor(out=t2[:], in0=mean2[:], in1=mean2[:],
                                    op=ALU.mult)
            nc.vector.scalar_tensor_tensor(
                out=var2[:], in0=sqs, scalar=1.0 / BN_CNT, in1=t2[:],
                op0=ALU.mult, op1=ALU.subtract)
            nc.scalar.activation(t2[:], var2[:], AF.Sqrt, bias=eps_t[:])
            nc.vector.reciprocal(var2[:], t2[:])
            nc.vector.tensor_tensor(out=sc2[:], in0=gam2[:], in1=var2[:],
                                    op=ALU.mult)
            nc.vector.tensor_tensor(out=t2[:], in0=mean2[:], in1=sc2[:],
                                    op=ALU.mult)
            nc.vector.tensor_tensor(out=sh2[:], in0=bet2[:], in1=t2[:],
                                    op=ALU.subtract)
            scale = [sc2[:, cb:cb + 1] for cb in range(CB)]
            shift = [sh2[:, cb:cb + 1] for cb in range(CB)]

            # normalized per-tap weights w = krnl*scale + shift, fp32
            wnorm = [[None] * CB for _ in range(NLOC)]
            for s in range(NLOC):
                for cb in range(CB):
                    wn = vpool.tile([128, FQ], dt, tag=f"wn{s}{cb}",
                                    name=f"wn{s}{cb}")
                    nc.vector.tensor_scalar(
                        out=wn[:], in0=krnl[s][cb][:],
                        scalar1=scale[cb], scalar2=shift[cb],
                        op0=ALU.mult, op1=ALU.add)
                    wnorm[s][cb] = wn

        # ---- depthwise convs + sigmoid + average ------------------------
        with tc.tile_pool(name="psz", bufs=4, space="PSUM") as psz:
            # all diag weights generated up front: unit 0's on DVE (they
            # gate the first PE taps), the rest on Pool before it gets
            # busy with sigmoid averaging
            diags = []
            for u in range(NU):
                s, cb = divmod(u, CB)
                wn = wnorm[s][cb]
                utaps = TAPS_LAST if u == NU - 1 else TAPS
                pe_taps = sorted(set(t for d in (1, 2, 3)
                                     for t in utaps[d]["pe"]))
                diag = {}
                for t in pe_taps:
                    dg = gpool.tile([128, 128], f16, tag="diag",
                                    name=f"dg{u}_{t}")
                    if u == 0:
                        nc.vector.tensor_scalar_mul(dg[:], ident[:],
                                                    wn[:, t:t + 1])
                    else:
                        nc.gpsimd.tensor_scalar_mul(dg[:], ident[:],
                                                    wn[:, t:t + 1])
                    diag[t] = dg
                diags.append(diag)
            for u in range(NU):
                s, cb = divmod(u, CB)
                wn = wnorm[s][cb]
                pad = pads[(s, cb)]
                utaps = TAPS_LAST if u == NU - 1 else TAPS
                diag = diags[u]
                # all Act taps up front so they never queue behind
                # earlier dilations' sigmoids in the in-order Act stream
                abufs_d = {}
                for d in (1, 2, 3):
                    abufs_d[d] = []
                    for t in utaps[d]["act"]:
                        dy, dx = tap_dydx(t, d)
                        ab = apool.tile([128, HW], f16, tag="ab",
                                        name=f"ab{u}_{d}_{t}")
                        nc.scalar.activation(
                            ab[:].rearrange("p (r c) -> p r c", c=W),
                            pwin(pad, 0, H, dy, dx), AF.Copy,
                            scale=wn[:, t:t + 1])
                        abufs_d[d].append(ab)
                sigs = []
                for d in (1, 2, 3):
                    cfg = utaps[d]
                    abufs = abufs_d[d]
                    # DVE taps: TS tmps, then TT chain into zd
                    zd = None
                    if cfg["dve"] or abufs:
                        zd = zpool.tile([128, HW], f16, tag="zd",
                                        name=f"zd{u}_{d}")
                        # last unit: half-image chaining so the first
                        # merges (and sigmoids) start half an image early
                        nhch = 2 if u == NU - 1 else 1
                        for hch in range(nhch):
                            hr = H // nhch
                            r0c = hch * hr
                            hsl = slice(r0c * W, (r0c + hr) * W)
                            tmps = []
                            for t in cfg["dve"]:
                                dy, dx = tap_dydx(t, d)
                                tm = tpool.tile([128, HW], f16, tag="tmp",
                                                name=f"tm{u}_{d}_{t}_{hch}")
                                nc.vector.tensor_scalar_mul(
                                    tm[:, hsl].rearrange(
                                        "p (r c) -> p r c", c=W),
                                    pwin(pad, r0c, hr, dy, dx),
                                    wn[:, t:t + 1])
                                tmps.append(tm)
                            terms = [(tm, hsl) for tm in tmps] +                                 [(ab, hsl) for ab in abufs]
                            nc.vector.tensor_tensor(
                                out=zd[:, hsl], in0=terms[0][0][:, hsl],
                                in1=terms[1][0][:, hsl], op=ALU.add)
                            for term, _ in terms[2:]:
                                nc.vector.tensor_tensor(
                                    out=zd[:, hsl], in0=zd[:, hsl],
                                    in1=term[:, hsl], op=ALU.add)
                    # PE taps + merge into PSUM quarters, sigmoid from PSUM
                    sg = spool.tile([128, HW], f16, tag="sig",
                                    name=f"sg{u}_{d}")
                    for q in range(NQ):
                        r0 = q * (H // NQ)
                        zq = psz.tile([128, QW], dt, tag="z",
                                      name=f"z{u}_{d}_{q}")
        # per 512-col window: its own start/stop group
                        n_grp = len(cfg["pe"]) + (1 if zd is not None else 0)
                        for ti, t in enumerate(cfg["pe"]):
                            dy, dx = tap_dydx(t, d)
                            for hh in range(2):
                                # trim rows whose shifted read is all pad
                                # zeros (only valid for non-start taps)
                                rb = r0 + hh * 8
                                lo = max(0, -dy - rb) if ti > 0 else 0
                                hi = 8 - (max(0, rb + 8 + dy - H)
                                          if ti > 0 else 0)
                                if lo >= hi:
                                    continue
                                nc.tensor.matmul(
                                    zq[:, hh * 512 + lo * W:
                                       hh * 512 + hi * W], diag[t][:],
                                    pwin(pad, rb + lo, hi - lo, dy, dx),
                                    start=(ti == 0), stop=(ti == n_grp - 1))
                        if zd is not None:
                            for hh in range(2):
                                nc.tensor.matmul(
                                    zq[:, hh * 512:(hh + 1) * 512], ident[:],
                                    zd[:, q * QW + hh * 512:
                                       q * QW + (hh + 1) * 512],
                                    start=False, stop=True)
                        nc.scalar.activation(sg[:, q * QW:(q + 1) * QW],
                                             zq[:], AF.Sigmoid)
                    sigs.append(sg)
                # average of the three sigmoids, fp16 out. GPSIMD for the
                # pipelined units; DVE for the last one (shorter drain tail)
                acc = opool.tile([128, HW], f16, tag="acc", name=f"acc{u}")
                outb = opool.tile([128, HW], f16, tag="outb", name=f"outb{u}")
                if u < NU - 1:
                    nc.gpsimd.tensor_tensor(out=acc[:], in0=sigs[0][:],
                                            in1=sigs[1][:], op=ALU.add)
                    nc.gpsimd.tensor_tensor(out=acc[:], in0=acc[:],
                                            in1=sigs[2][:], op=ALU.add)
                    nc.gpsimd.tensor_scalar_mul(outb[:], acc[:], 1.0 / 3.0)
                    nc.sync.dma_start(
                        out_d.ap()[s, cb * 128:(cb + 1) * 128],
                        outb[:].rearrange("p (r c) -> p r c", c=W))
                else:
                    # last unit: quarter-chunked average + eager DMA; the
                    # early quarters ride on Pool, only the final one sits
                    # on the drain-critical DVE
                    for q in range(NQ):
                        ql = slice(q * QW, (q + 1) * QW)
                        nc.vector.tensor_tensor(out=acc[:, ql],
                                                in0=sigs[0][:, ql],
                                                in1=sigs[1][:, ql], op=ALU.add)
                        nc.vector.tensor_tensor(out=acc[:, ql],
                                                in0=acc[:, ql],
                                                in1=sigs[2][:, ql], op=ALU.add)
                        nc.vector.tensor_scalar_mul(outb[:, ql], acc[:, ql],
                                                    1.0 / 3.0)
                        nc.sync.dma_start(
                            out_d.ap()[s, cb * 128:(cb + 1) * 128,
                                       q * 16:(q + 1) * 16],
                            outb[:, ql].rearrange("p (r c) -> p r c", c=W))


def _build(bias_general=False):
    nc = bacc.Bacc("TRN2", debug=False, num_devices=N_CORES,
                   target_bir_lowering=False)
    xh_d = nc.dram_tensor("xh", [NLOC, C, H, W], f16, kind="ExternalInput")
    w_d = nc.dram_tensor("wkqt", [C, FKQ], f16, kind="ExternalInput")
    bias_d = nc.dram_tensor("biaskq", [FKQ], dt, kind="ExternalInput")
    g_d = nc.dram_tensor("gamma", [C], dt, kind="ExternalInput")
    b_d = nc.dram_tensor("beta", [C], dt, kind="ExternalInput")
    out_d = nc.dram_tensor("out", [NLOC, C, H, W], f16, kind="ExternalOutput")
    with tile.TileContext(nc) as tc:
        _body(nc, tc, (xh_d, w_d, bias_d, g_d, b_d, out_d), bias_general)
    nc.compile()
    return nc


_nc_cache = {}
last_results = None


def kernel(x, wk, bk, wq, bq, gamma, beta):
    global last_results
    # biases are zeros in this problem's setup; the zero-bias variant skips
    # their (exact) folding. The general variant handles nonzero biases.
    bias_general = bool(np.any(np.asarray(bk)) or np.any(np.asarray(bq)))
    if bias_general not in _nc_cache:
        _nc_cache[bias_general] = _build(bias_general)
    nc = _nc_cache[bias_general]
    x = np.ascontiguousarray(x, dtype=np.float32)
    xh = x.astype(np.float16)
    wkqt = np.concatenate(
        [np.asarray(wk, np.float32).T, np.asarray(wq, np.float32).T],
        axis=1).astype(np.float16)  # [C, 265]
    biaskq = np.concatenate(
        [np.asarray(bk, np.float32), np.asarray(bq, np.float32)])
    in_maps = []
    for c in range(N_CORES):
        sl = slice(c * NLOC, (c + 1) * NLOC)
        in_maps.append({
            "xh": np.ascontiguousarray(xh[sl]),
            "wkqt": np.ascontiguousarray(wkqt),
            "biaskq": np.ascontiguousarray(biaskq, np.float32),
            "gamma": np.ascontiguousarray(gamma, np.float32),
            "beta": np.ascontiguousarray(beta, np.float32),
        })
    res = bass_utils.run_bass_kernel_spmd(
        nc, in_maps, core_ids=list(range(N_CORES)))
    last_results = res
    out = np.concatenate([res.results[c]["out"] for c in range(N_CORES)],
                         axis=0)
    return out.astype(np.float32)


# revision 74
# speedup vs baseline: 1.3005x; 1.0096x over previous
"""Trainium2 Bass kernel for nn_CaC_50637664420271.

Computes, for x:[16,256,64,64]:
  feat_k = wk @ x + bk  (1x1 conv), feat_q = wq @ x + bq
  krnl[n,c,3,3] = bmm(feat_k, feat_q^T)  -> BatchNorm (train stats) ->
  out = mean_d sigmoid(depthwise_conv(x, krnl, dilation=d)), d in {1,2,3}

Sharding: pure data-parallel over batch (2 samples / core, 8 cores), with a
tiny AllReduce of per-channel (sum, sumsq) of krnl for the BN batch stats.

Single-product fp16 pipeline: x is converted to fp16 on the host and loaded
once per unit as a zero-padded image; features stream from the same padded
tile (strided stationary chunks), so x is read from HBM exactly once. The
depthwise conv splits its 9 taps per dilation across the TensorEngine
(diag-weight fp16 matmuls into PSUM), the VectorEngine (tensor_scalar at 4x +
tensor_tensor at 2x on fp16), and the ScalarEngine (copy-with-scale taps);
DVE partials merge into PSUM via identity matmuls, sigmoid reads PSUM
directly, and GPSIMD averages the three sigmoids and writes fp16 output.
"""
import os
import numpy as np

import concourse.bass as bass
import concourse.bacc as bacc
import concourse.tile as tile
import concourse.mybir as mybir
from concourse import bass_utils

N_CORES = 8
NLOC = 2            # samples per core
C = 256
H = W = 64
HW = H * W          # 4096
S = 3
PAD = 3
WP = W + 2 * PAD    # padded row width 70
HP = H + 2 * PAD
PSZ = WP * HP       # 70*70 = 4900 padded image size
CB = C // 128       # channel blocks per sample (2)
NU = NLOC * CB      # units per core (4)
PB = HW // 128      # pixel blocks per sample (32)
FQ = S * S          # 9
FKQ = C + FQ        # 265 fused feature columns
BN_EPS = 1e-5
BN_CNT = 16 * FQ    # 144 elements per channel in BN stats

NQ = 8              # psum chunks per dilation image
QW = HW // NQ       # 1024 px per quarter (16 rows)

# per-dilation engine split of the 9 taps; center (4) first so every PSUM
# window's start-tap has full row coverage (shifted taps trim zero rows)
TAPS = {
    1: {"pe": (4, 0, 2, 6, 8), "dve": (1, 3, 5), "act": (7,)},
    2: {"pe": (4, 0, 2, 6, 8), "dve": (1, 3, 5), "act": (7,)},
    3: {"pe": (4, 0, 2, 6, 8), "dve": (1, 3, 5), "act": (7,)},
}
TAPS_LAST = TAPS

dt = mybir.dt.float32
f16 = mybir.dt.float16
ALU = mybir.AluOpType
AF = mybir.ActivationFunctionType
AX = mybir.AxisListType


def tap_dydx(t, d):
    return d * (t // S - 1), d * (t % S - 1)


def _body(nc, tc, tens, bias_general):
    xh_d, w_d, bias_d, g_d, b_d, out_d = tens
    with tc.tile_pool(name="const", bufs=1) as cpool, \
         tc.tile_pool(name="pimg", bufs=4) as ppool, \
         tc.tile_pool(name="fbp", bufs=6) as fbpool, \
         tc.tile_pool(name="tmp", bufs=3) as tpool, \
         tc.tile_pool(name="zd", bufs=2) as zpool, \
         tc.tile_pool(name="ab", bufs=3) as apool, \
         tc.tile_pool(name="sig", bufs=3) as spool, \
         tc.tile_pool(name="acc", bufs=2) as opool, \
         tc.tile_pool(name="diag", bufs=20) as gpool, \
         tc.tile_pool(name="small", bufs=1) as vpool, \
         tc.tile_pool(name="work", bufs=4) as wpool, \
         tc.tile_pool(name="dram", bufs=2, space="DRAM") as dpool:

        # ---- constants / weights ----------------------------------------
        ident_d = nc.inline_tensor(np.eye(128).astype(np.float16),
                                   name="identh")
        ident = cpool.tile([128, 128], f16, tag="ident")
        nc.sync.dma_start(ident[:], ident_d.ap())

        wkq = []
        for ki in range(CB):
            t = cpool.tile([128, FKQ], f16, tag=f"wkq{ki}", name=f"wkq{ki}")
            nc.sync.dma_start(t[:], w_d.ap()[ki * 128:(ki + 1) * 128, :])
            wkq.append(t)

        ones_r32 = cpool.tile([1, 128], dt, tag="ones32")
        nc.vector.memset(ones_r32[:], 1.0)
        bias_r = cpool.tile([1, FKQ], dt, tag="biasr")
        nc.sync.dma_start(bias_r[:], bias_d.ap().rearrange("(p f) -> p f", p=1))
        gam, bet = [], []
        for cb in range(CB):
            gt = cpool.tile([128, 1], dt, tag=f"g{cb}", name=f"g{cb}")
            bt = cpool.tile([128, 1], dt, tag=f"b{cb}", name=f"b{cb}")
            nc.sync.dma_start(
                gt[:], g_d.ap().rearrange("(p f) -> p f", f=1)[cb * 128:(cb + 1) * 128, :])
            nc.sync.dma_start(
                bt[:], b_d.ap().rearrange("(p f) -> p f", f=1)[cb * 128:(cb + 1) * 128, :])
            gam.append(gt)
            bet.append(bt)

        def pwin(pt, r0, nr, dy, dx):
            g = pt[:].rearrange("p (r c) -> p r c", c=WP)
            return g[:, PAD + r0 + dy:PAD + r0 + dy + nr,
                     PAD + dx:PAD + dx + W]

        # ---- images: contiguous copies first (feature stationaries, so
        # features can start early), padded fp16 images for conv windows --
        xs = {}
        for u in range(NU):
            s, cb = divmod(u, CB)
            xs[(s, cb)] = ppool.tile([128, HW], f16, tag="xs",
                                     name=f"xs{u}")
        # quarter-granular loads, cb0/cb1 interleaved per sample, so the
        # first feature matmuls start ~2us in instead of waiting for the
        # whole 8MB
        for s in range(NLOC):
            for q in range(4):
                for cb in range(CB):
                    qs = slice(q * (HW // 4), (q + 1) * (HW // 4))
                    nc.sync.dma_start(
                        xs[(s, cb)][:, qs],
                        xh_d.ap()[s, cb * 128:(cb + 1) * 128].rearrange(
                            "p r c -> p (r c)")[:, qs])
        pads = {}
        for u in range(NU):
            s, cb = divmod(u, CB)
            t = ppool.tile([128, PSZ], f16, tag="pimg", name=f"pad{u}")
            pg = t[:].rearrange("p (r c) -> p r c", c=WP)
            nc.gpsimd.memset(t[:, 0:PAD * WP + PAD], 0.0)
            nc.gpsimd.memset(t[:, PSZ - PAD * WP - PAD:PSZ], 0.0)
            nc.gpsimd.memset(pg[:, PAD:PAD + H, 0:PAD], 0.0)
            nc.gpsimd.memset(pg[:, PAD:PAD + H, PAD + W:WP], 0.0)
            pads[(s, cb)] = t

        # ---- bias broadcast [128, 265] fp16 via ones-matmul --------------
        with tc.tile_pool(name="psf", bufs=5, space="PSUM") as psf, \
             tc.tile_pool(name="psk", bufs=1, space="PSUM") as psk:
            # ---- features + per-sample kernel bmm -----------------------
            # fbT[p, c'] = sum_c x[c,p] wkq[c,c'] + bias  (pixel-major)
            # krnl[c, t] = sum_p fbT[p, c] * fbT[p, 256+t]
            krnl = [[vpool.tile([128, FQ], dt, tag=f"krnl{s}{cb}",
                                name=f"krnl{s}{cb}")
                     for cb in range(CB)] for s in range(NLOC)]
            prts = [vpool.tile([128, 4], dt, tag=f"prt{cb}", name=f"prt{cb}")
                    for cb in range(CB)]
            # bias broadcast [128, 265] fp16, folded into the evac TT
            bbp = psf.tile([128, FKQ], dt, tag="fb", name="bbp")
            nc.tensor.matmul(bbp[:], ones_r32[:], bias_r[:],
                             start=True, stop=True)
            bias_bc = cpool.tile([128, FKQ], f16, tag="bbc")
            nc.vector.tensor_copy(bias_bc[:], bbp[:])

            for s in range(NLOC):
                kps = [psk.tile([128, FQ], dt, tag=f"kp{cb}",
                                name=f"kp{s}{cb}")
                       for cb in range(CB)]
                fbs = [None] * PB

                def bmm(pb):
                    fb = fbs[pb]
                    for cb in range(CB):
                        nc.tensor.matmul(kps[cb][:],
                                         fb[:, cb * 128:(cb + 1) * 128],
                                         fb[:, C:C + FQ],
                                         start=(pb == 0), stop=(pb == PB - 1))

                for pb in range(PB):
                    fp = psf.tile([128, FKQ], dt, tag="fb", name=f"feat{s}{pb}")
                    for cb in range(CB):
                        stat = xs[(s, cb)][:, pb * 128:(pb + 1) * 128]
                        nc.tensor.matmul(fp[:], stat, wkq[cb][:],
                                         start=(cb == 0), stop=(cb == CB - 1))
                    fb = fbpool.tile([128, FKQ], f16, tag="fbs",
                                     name=f"fb{s}{pb}")
                    if not bias_general:
                        # biases known-zero: plain alternating evacuation
                        if pb % 2 == 0:
                            nc.vector.tensor_copy(fb[:], fp[:])
                        else:
                            nc.scalar.copy(fb[:], fp[:])
                    elif pb % 2 == 0:
                        nc.vector.tensor_tensor(out=fb[:], in0=fp[:],
                                                in1=bias_bc[:], op=ALU.add)
                    else:
                        nc.scalar.copy(fb[:], fp[:])
                        nc.vector.tensor_tensor(out=fb[:], in0=fb[:],
                                                in1=bias_bc[:], op=ALU.add)
                    fbs[pb] = fb
                    # bmm lags two blocks so the evac round-trip never
                    # stalls the PE
                    if pb >= 2:
                        bmm(pb - 2)
                bmm(PB - 2)
                bmm(PB - 1)
                for cb in range(CB):
                    nc.vector.tensor_copy(krnl[s][cb][:], kps[cb][:])
                    # per-sample stat partials overlap the next sample
                    nc.vector.tensor_reduce(prts[cb][:, s:s + 1],
                                            krnl[s][cb][:], AX.X, ALU.add)
                    tmp = wpool.tile([128, FQ], dt, tag="sq", name="sq")
                    nc.vector.tensor_tensor(out=tmp[:], in0=krnl[s][cb][:],
                                            in1=krnl[s][cb][:], op=ALU.mult)
                    nc.vector.tensor_reduce(prts[cb][:, 2 + s:3 + s], tmp[:],
                                            AX.X, ALU.add)

            # ---- BN stats + AllReduce -----------------------------------
            # single [128, 4] tile: cols (sum0, sq0, sum1, sq1) per cb pair
            stt = vpool.tile([128, 4], dt, tag="stt", name="stt")
            for cb in range(CB):
                nc.vector.tensor_tensor(out=stt[:, 2 * cb:2 * cb + 1],
                                        in0=prts[cb][:, 0:1],
                                        in1=prts[cb][:, 1:2], op=ALU.add)
                nc.vector.tensor_tensor(out=stt[:, 2 * cb + 1:2 * cb + 2],
                                        in0=prts[cb][:, 2:3],
                                        in1=prts[cb][:, 3:4], op=ALU.add)

            ib = dpool.tile([128, CB * 2], dt)
            ob = dpool.tile([128, CB * 2], dt)
            nc.sync.dma_start(ib[:], stt[:])
            # pad interiors built on-chip (4x-mode strided fp16 copies)
            # while DVE idles during the collective; this also keeps both
            # the DMA queue and the Pool stream (which issues the
            # collective) clear
            for u in range(NU):
                s, cb = divmod(u, CB)
                pg2 = pads[(s, cb)][:].rearrange("p (r c) -> p r c", c=WP)
                nc.vector.tensor_copy(
                    pg2[:, PAD:PAD + H, PAD:PAD + W],
                    xs[(s, cb)][:].rearrange("p (r c) -> p r c", c=W))
            if os.environ.get("PROF_NO_CC"):
                nc.gpsimd.dma_start(ob[:], ib[:])
            else:
                nc.gpsimd.collective_compute(
                    "AllReduce", ALU.add,
                    replica_groups=[list(range(N_CORES))],
                    ins=[ib.opt()], outs=[ob.opt()])

            glt = vpool.tile([128, 4], dt, tag="glt", name="glt")
            nc.sync.dma_start(glt[:], ob[:])
            eps_t = vpool.tile([128, 1], dt, tag="eps")
            nc.vector.memset(eps_t[:], BN_EPS)
            # vectorized BN math over both channel blocks: [128, 2] views
            g3 = glt[:].rearrange("p (c t) -> p c t", t=2)
            sums = g3[:, :, 0]
            sqs = g3[:, :, 1]
            gam2 = cpool.tile([128, CB], dt, tag="gam2")
            bet2 = cpool.tile([128, CB], dt, tag="bet2")
            for cb in range(CB):
                nc.vector.tensor_copy(gam2[:, cb:cb + 1], gam[cb][:])
                nc.vector.tensor_copy(bet2[:, cb:cb + 1], bet[cb][:])
            mean2 = vpool.tile([128, CB], dt, tag="mean2", name="mean2")
            var2 = vpool.tile([128, CB], dt, tag="var2", name="var2")
            sc2 = vpool.tile([128, CB], dt, tag="sc2", name="sc2")
            sh2 = vpool.tile([128, CB], dt, tag="sh2", name="sh2")
            t2 = wpool.tile([128, CB], dt, tag="bnt", name="bnt")
            nc.vector.tensor_scalar_mul(mean2[:], sums, 1.0 / BN_CNT)
            nc.vector.tensor_tensor(out=t2[:], in0=mean2[:], in1=mean2[:],
                                    op=ALU.mult)
            nc.vector.scalar_tensor_tensor(
                out=var2[:], in0=sqs, scalar=1.0 / BN_CNT, in1=t2[:],
                op0=ALU.mult, op1=ALU.subtract)
            nc.scalar.activation(t2[:], var2[:], AF.Sqrt, bias=eps_t[:])
            nc.vector.reciprocal(var2[:], t2[:])
            nc.vector.tensor_tensor(out=sc2[:], in0=gam2[:], in1=var2[:],
                                    op=ALU.mult)
            nc.vector.tensor_tensor(out=t2[:], in0=mean2[:], in1=sc2[:],
                                    op=ALU.mult)
            nc.vector.tensor_tensor(out=sh2[:], in0=bet2[:], in1=t2[:],
                                    op=ALU.subtract)
            scale = [sc2[:, cb:cb + 1] for cb in range(CB)]
            shift = [sh2[:, cb:cb + 1] for cb in range(CB)]

            # normalized per-tap weights w = krnl*scale + shift, fp32
            wnorm = [[None] * CB for _ in range(NLOC)]
            for s in range(NLOC):
                for cb in range(CB):
                    wn = vpool.tile([128, FQ], dt, tag=f"wn{s}{cb}",
                                    name=f"wn{s}{cb}")
                    nc.vector.tensor_scalar(
                        out=wn[:], in0=krnl[s][cb][:],
                        scalar1=scale[cb], scalar2=shift[cb],
                        op0=ALU.mult, op1=ALU.add)
                    wnorm[s][cb] = wn

        # ---- depthwise convs + sigmoid + average ------------------------
        with tc.tile_pool(name="psz", bufs=8, space="PSUM") as psz:
            # all diag weights generated up front: unit 0's on DVE (they
            # gate the first PE taps), the rest on Pool before it gets
            # busy with sigmoid averaging
            diags = []
            for u in range(NU):
                s, cb = divmod(u, CB)
                wn = wnorm[s][cb]
                utaps = TAPS_LAST if u == NU - 1 else TAPS
                pe_taps = sorted(set(t for d in (1, 2, 3)
                                     for t in utaps[d]["pe"]))
                diag = {}
                for t in pe_taps:
                    dg = gpool.tile([128, 128], f16, tag="diag",
                                    name=f"dg{u}_{t}")
                    if u == 0:
                        nc.vector.tensor_scalar_mul(dg[:], ident[:],
                                                    wn[:, t:t + 1])
                    else:
                        nc.gpsimd.tensor_scalar_mul(dg[:], ident[:],
                                                    wn[:, t:t + 1])
                    diag[t] = dg
                diags.append(diag)
            for u in range(NU):
                s, cb = divmod(u, CB)
                wn = wnorm[s][cb]
                pad = pads[(s, cb)]
                utaps = TAPS_LAST if u == NU - 1 else TAPS
                diag = diags[u]
                # all Act taps up front so they never queue behind
                # earlier dilations' sigmoids in the in-order Act stream
                abufs_d = {}
                for d in (1, 2, 3):
                    abufs_d[d] = []
                    for t in utaps[d]["act"]:
                        dy, dx = tap_dydx(t, d)
                        ab = apool.tile([128, HW], f16, tag="ab",
                                        name=f"ab{u}_{d}_{t}")
                        nc.scalar.activation(
                            ab[:].rearrange("p (r c) -> p r c", c=W),
                            pwin(pad, 0, H, dy, dx), AF.Copy,
                            scale=wn[:, t:t + 1])
                        abufs_d[d].append(ab)
                sigs = []
                for d in (1, 2, 3):
                    cfg = utaps[d]
                    abufs = abufs_d[d]
                    # DVE taps: TS tmps, then TT chain into zd
                    zd = None
                    if cfg["dve"] or abufs:
                        zd = zpool.tile([128, HW], f16, tag="zd",
                                        name=f"zd{u}_{d}")
                        # last unit: half-image chaining so the first
                        # merges (and sigmoids) start half an image early
                        nhch = 2 if u == NU - 1 else 1
                        for hch in range(nhch):
                            hr = H // nhch
                            r0c = hch * hr
                            hsl = slice(r0c * W, (r0c + hr) * W)
                            tmps = []
                            for t in cfg["dve"]:
                                dy, dx = tap_dydx(t, d)
                                tm = tpool.tile([128, HW], f16, tag="tmp",
                                                name=f"tm{u}_{d}_{t}_{hch}")
                                nc.vector.tensor_scalar_mul(
                                    tm[:, hsl].rearrange(
                                        "p (r c) -> p r c", c=W),
                                    pwin(pad, r0c, hr, dy, dx),
                                    wn[:, t:t + 1])
                                tmps.append(tm)
                            terms = [(tm, hsl) for tm in tmps] +                                 [(ab, hsl) for ab in abufs]
                            nc.vector.tensor_tensor(
                                out=zd[:, hsl], in0=terms[0][0][:, hsl],
                                in1=terms[1][0][:, hsl], op=ALU.add)
                            for term, _ in terms[2:]:
                                nc.vector.tensor_tensor(
                                    out=zd[:, hsl], in0=zd[:, hsl],
                                    in1=term[:, hsl], op=ALU.add)
                    # PE taps + merge into PSUM quarters, sigmoid from PSUM
                    sg = spool.tile([128, HW], f16, tag="sig",
                                    name=f"sg{u}_{d}")
                    for q in range(NQ):
                        r0 = q * (H // NQ)
                        zq = psz.tile([128, QW], dt, tag="z",
                                      name=f"z{u}_{d}_{q}")
        # per 512-col window: its own start/stop group
                        n_grp = len(cfg["pe"]) + (1 if zd is not None else 0)
                        for ti, t in enumerate(cfg["pe"]):
                            dy, dx = tap_dydx(t, d)
                            for hh in range(QW // 512):
                                # trim rows whose shifted read is all pad
                                # zeros (only valid for non-start taps)
                                rb = r0 + hh * 8
                                lo = max(0, -dy - rb) if ti > 0 else 0
                                hi = 8 - (max(0, rb + 8 + dy - H)
                                          if ti > 0 else 0)
                                if lo >= hi:
                                    continue
                                nc.tensor.matmul(
                                    zq[:, hh * 512 + lo * W:
                                       hh * 512 + hi * W], diag[t][:],
                                    pwin(pad, rb + lo, hi - lo, dy, dx),
                                    start=(ti == 0), stop=(ti == n_grp - 1))
                        if zd is not None:
                            for hh in range(QW // 512):
                                nc.tensor.matmul(
                                    zq[:, hh * 512:(hh + 1) * 512], ident[:],
                                    zd[:, q * QW + hh * 512:
                                       q * QW + (hh + 1) * 512],
                                    start=False, stop=True)
                        nc.scalar.activation(sg[:, q * QW:(q + 1) * QW],
                                             zq[:], AF.Sigmoid)
                    sigs.append(sg)
                # average of the three sigmoids, fp16 out. GPSIMD for the
                # pipelined units; DVE for the last one (shorter drain tail)
                acc = opool.tile([128, HW], f16, tag="acc", name=f"acc{u}")
                outb = opool.tile([128, HW], f16, tag="outb", name=f"outb{u}")
                if u < NU - 1:
                    nc.gpsimd.tensor_tensor(out=acc[:], in0=sigs[0][:],
                                            in1=sigs[1][:], op=ALU.add)
                    nc.gpsimd.tensor_tensor(out=acc[:], in0=acc[:],
                                            in1=sigs[2][:], op=ALU.add)
                    nc.gpsimd.tensor_scalar_mul(outb[:], acc[:], 1.0 / 3.0)
                    nc.sync.dma_start(
                        out_d.ap()[s, cb * 128:(cb + 1) * 128],
                        outb[:].rearrange("p (r c) -> p r c", c=W))
                else:
                    # last unit: quarter-chunked average + eager DMA; the
                    # early quarters ride on Pool, only the final one sits
                    # on the drain-critical DVE
                    for q in range(NQ):
                        ql = slice(q * QW, (q + 1) * QW)
                        nc.vector.tensor_tensor(out=acc[:, ql],
                                                in0=sigs[0][:, ql],
                                                in1=sigs[1][:, ql], op=ALU.add)
                        nc.vector.tensor_tensor(out=acc[:, ql],
                                                in0=acc[:, ql],
                                                in1=sigs[2][:, ql], op=ALU.add)
                        nc.vector.tensor_scalar_mul(outb[:, ql], acc[:, ql],
                                                    1.0 / 3.0)
                        nc.sync.dma_start(
                            out_d.ap()[s, cb * 128:(cb + 1) * 128,
                                       q * (H // NQ):(q + 1) * (H // NQ)],
                            outb[:, ql].rearrange("p (r c) -> p r c", c=W))


def _build(bias_general=False):
    nc = bacc.Bacc("TRN2", debug=False, num_devices=N_CORES,
                   target_bir_lowering=False)
    xh_d = nc.dram_tensor("xh", [NLOC, C, H, W], f16, kind="ExternalInput")
    w_d = nc.dram_tensor("wkqt", [C, FKQ], f16, kind="ExternalInput")
    bias_d = nc.dram_tensor("biaskq", [FKQ], dt, kind="ExternalInput")
    g_d = nc.dram_tensor("gamma", [C], dt, kind="ExternalInput")
    b_d = nc.dram_tensor("beta", [C], dt, kind="ExternalInput")
    out_d = nc.dram_tensor("out", [NLOC, C, H, W], f16, kind="ExternalOutput")
    with tile.TileContext(nc) as tc:
        _body(nc, tc, (xh_d, w_d, bias_d, g_d, b_d, out_d), bias_general)
    nc.compile()
    return nc


_nc_cache = {}
last_results = None


def kernel(x, wk, bk, wq, bq, gamma, beta):
    global last_results
    # biases are zeros in this problem's setup; the zero-bias variant skips
    # their (exact) folding. The general variant handles nonzero biases.
    bias_general = bool(np.any(np.asarray(bk)) or np.any(np.asarray(bq)))
    if bias_general not in _nc_cache:
        _nc_cache[bias_general] = _build(bias_general)
    nc = _nc_cache[bias_general]
    x = np.ascontiguousarray(x, dtype=np.float32)
    xh = x.astype(np.float16)
    wkqt = np.concatenate(
        [np.asarray(wk, np.float32).T, np.asarray(wq, np.float32).T],
        axis=1).astype(np.float16)  # [C, 265]
    biaskq = np.concatenate(
        [np.asarray(bk, np.float32), np.asarray(bq, np.float32)])
    in_maps = []
    for c in range(N_CORES):
        sl = slice(c * NLOC, (c + 1) * NLOC)
        in_maps.append({
            "xh": np.ascontiguousarray(xh[sl]),
            "wkqt": np.ascontiguousarray(wkqt),
            "biaskq": np.ascontiguousarray(biaskq, np.float32),
            "gamma": np.ascontiguousarray(gamma, np.float32),
            "beta": np.ascontiguousarray(beta, np.float32),
        })
    res = bass_utils.run_bass_kernel_spmd(
        nc, in_maps, core_ids=list(range(N_CORES)))
    last_results = res
    out = np.concatenate([res.results[c]["out"] for c in range(N_CORES)],
                         axis=0)
    return out.astype(np.float32)
